# revision 64
# baseline (speedup 1.0000x reference)
"""Trainium2 Bass kernel for a dense transformer block (B=4,T=2048,C=1024,H=16).

Sharding: 8 cores, zero collectives. Core i handles batch i//2; its 1024
query tokens are four 256-token chunks, one per causal "slot" s=0..3 at
tokens [512s+256*(i%2), +256).  Slot s attends to exactly 512(s+1) keys, so
the uniform SPMD program does the causally-minimal score work (40 key-tile
passes per core vs 48 for a 2x512 split); per-core host-built multiplicative
masks handle the diagonal and the core asymmetry.  All sharding on the host;
the program is identical on every core, only input data differs.

Per-core dataflow (tokens-on-free-axis for all matmul operands):
  x ships bf16.  LN1 (bn_stats) -> h1 bf16 -> PE-transpose (no DRAM bounce)
  -> h1T [C, tok] in SBUF.  QKV bf16: kT [H*64, tok] (2 heads per tile); V
  natural [tok, H*65] with a fused ones column so PV also produces the
  softmax denominator; queries go to qTz [128, slot, 512], a zero-padded
  block-diagonal layout so ONE [128,512] matmul per key tile scores both
  stacked heads (full-rate, bank-exclusive PSUM).  wq is preloaded into the
  h2T tile (same shape, dead until mid-attention).  Scores are pre-
  transposed sT[tk, tq] so softmax needs no transpose of P and no max
  subtraction; exp runs on ACT straight from PSUM with the 1/sqrt(hd) scale
  fused.  V tiles 8-15 are deferred as PE fillers inside attention slots
  0-1.  After each slot, the proj for its two 128-token tiles runs fused:
  out-proj matmul + residual (+bias) in bf16, then LN2 + PE-transpose into
  h2T immediately - no separate LN2 phase.  FFN is jb-outer (weights load
  ONCE, bf16, double-buffered), jb0 initializes the accumulator with the
  prefetched x2 residual; all matmuls bf16 with fp32 PSUM accumulation.
  Measured (cost-model timeline): ~647us/core; PE busy ~82%.
"""

import sys
import numpy as np

for _p in ("/opt/trn_rl_repo", "/root/.axon_site/_ro/trn_rl_repo"):
    if _p not in sys.path:
        sys.path.append(_p)

import ml_dtypes  # noqa: E402
import concourse.bass as bass  # noqa: E402
import concourse.bacc as bacc  # noqa: E402
import concourse.tile as tile  # noqa: E402
from concourse import mybir  # noqa: E402
from concourse.bass_utils import run_bass_kernel_spmd  # noqa: E402
from concourse.masks import make_identity  # noqa: E402

B, T, C, H, HD = 4, 2048, 1024, 16, 64
NCORES = 8
EPS = 1e-5
F32 = mybir.dt.float32
F32R = mybir.dt.float32r
BF16 = mybir.dt.bfloat16
AF = mybir.ActivationFunctionType
ALU = mybir.AluOpType

_CACHE = {}

def _emit_body(nc, tc, io, ln1_triv, ln2_triv):
    # ---------------- long-lived pools ----------------
    def pool(name, bufs, space="SBUF"):
        cm = tc.tile_pool(name=name, bufs=bufs, space=space)
        p = cm.__enter__()
        return cm, p

    cm_singles, singles = pool("singles", 1)
    cm_ln, ln_pool = pool("ln", 3)
    cm_stat, stat_pool = pool("stat", 4)
    cm_small, small = pool("small", 2)
    cm_dram, dram = pool("dram", 1, "DRAM")

    eps_t = singles.tile([128, 1], F32, name="eps")
    nc.vector.memset(eps_t, EPS)
    ident_bf = singles.tile([128, 128], BF16, name="ident_bf")
    make_identity(nc, ident_bf)
    b1t_sb = singles.tile([128, 32], F32, name="b1t_sb")
    nc.gpsimd.dma_start(out=b1t_sb, in_=io["b1t"])

    def bcast_ap(dram_ap):
        # [1024] dram vector -> [128,1024] partition-broadcast AP
        return bass.AP(
            tensor=dram_ap.tensor,
            offset=dram_ap.offset,
            ap=[[0, 128]] + list(dram_ap.ap),
        )

    bproj_sb = singles.tile([128, 1024], F32, name="bproj_sb")
    nc.gpsimd.dma_start(out=bproj_sb, in_=bcast_ap(io["b_proj"]))

    g1_sb = bb1_sb = g2_sb = bb2_sb = None
    if not ln1_triv:
        g1_sb = singles.tile([128, 1024], F32, name="g1_sb")
        nc.gpsimd.dma_start(out=g1_sb, in_=bcast_ap(io["ln1_g"]))
        bb1_sb = singles.tile([128, 1024], F32, name="bb1_sb")
        nc.gpsimd.dma_start(out=bb1_sb, in_=bcast_ap(io["ln1_b"]))
    if not ln2_triv:
        g2_sb = singles.tile([128, 1024], F32, name="g2_sb")
        nc.gpsimd.dma_start(out=g2_sb, in_=bcast_ap(io["ln2_g"]))
        bb2_sb = singles.tile([128, 1024], F32, name="bb2_sb")
        nc.gpsimd.dma_start(out=bb2_sb, in_=bcast_ap(io["ln2_b"]))

    # ---------------- LayerNorm helpers ----------------
    def ln_apply(xt, out_ap, trivial, g_sb, b_sb):
        st = stat_pool.tile([128, 2, 6], F32, tag="bnst", name="bnst")
        for sg in range(2):
            nc.vector.bn_stats(out=st[:, sg, :], in_=xt[:, sg * 512:(sg + 1) * 512])
        mv = stat_pool.tile([128, 2], F32, tag="bnmv", name="bnmv")
        nc.vector.bn_aggr(out=mv, in_=st)
        std = stat_pool.tile([128, 1], F32, tag="bnsd", name="bnsd")
        nc.scalar.activation(out=std, in_=mv[:, 1:2], func=AF.Sqrt, bias=eps_t,
                             scale=1.0)
        rstd = stat_pool.tile([128, 1], F32, tag="bnrs", name="bnrs")
        nc.vector.reciprocal(out=rstd, in_=std)
        if trivial:
            nc.vector.tensor_scalar(
                out=out_ap, in0=xt, scalar1=mv[:, 0:1], scalar2=rstd,
                op0=ALU.subtract, op1=ALU.mult)
        else:
            tmp = ln_pool.tile([128, 1024], F32, tag="lnx", name="lntmp")
            nc.vector.tensor_scalar(
                out=tmp, in0=xt, scalar1=mv[:, 0:1], scalar2=rstd,
                op0=ALU.subtract, op1=ALU.mult)
            nc.vector.tensor_mul(out=tmp, in0=tmp, in1=g_sb)
            nc.vector.tensor_add(out=out_ap, in0=tmp, in1=b_sb)

    def ln_tile(x_src_rows, out_ap, trivial, g_sb, b_sb):
        xt = ln_pool.tile([128, 1024], BF16, tag="lnx", name="lnx")
        nc.sync.dma_start(out=xt, in_=x_src_rows)
        ln_apply(xt, out_ap, trivial, g_sb, b_sb)

    # long-lived result pools, opened bottom-of-stack (LIFO discipline)
    cm_h2t, h2t_pool = pool("h2t", 1)
    h2T = h2t_pool.tile([128, 8, 1024], BF16, name="h2T")
    cm_kt, kt_pool = pool("kt", 8)
    cm_v, v_pool = pool("v", 16)
    cm_qt, qt_pool = pool("qt", 8)
    kT = [kt_pool.tile([128, 2048], BF16, tag="kt", name="kt") for _ in range(8)]
    Vt = [v_pool.tile([128, 16, 65], BF16, tag="vt", name="vt") for _ in range(16)]
    # qTz[p]: [128, 4(slot), 512] zero-padded block-diagonal queries: rows
    # 0-63 hold head-e0 q in cols 0-255, rows 64-127 head-e1 q in cols
    # 256-511.  One score matmul per key tile then covers both heads with
    # the full 128-partition contraction.
    qTz = [qt_pool.tile([128, 4, 512], BF16, tag="qt", name="qtz")
           for _ in range(8)]
    for p in range(8):
        nc.gpsimd.memset(qTz[p], 0.0)
    x2d = dram.tile([1024, 1024], BF16, name="x2d")

    cm_masks, masks_pool = pool("masks", 1)
    masks_sb = masks_pool.tile([128, 16, 256], BF16, name="masks_sb")
    nc.gpsimd.dma_start(out=masks_sb, in_=io["masks"])

    # ---------------- Phase 1+2: LN1 -> PE-transpose -> QKV -----------------
    # weight loads are emitted AFTER the first x tiles: the DMA engine is a
    # serial resource and the first LN tiles are on the critical path
    cm_wqkv, wqkv_pool = pool("wqkv", 2)
    wkB = wqkv_pool.tile([128, 8, 1024], BF16, tag="w", name="wkB")
    wvB = wqkv_pool.tile([128, 8, 1024], BF16, tag="w", name="wvB")

    cm_h1t, h1t_pool = pool("h1t", 2)
    cm_pst, ps_tr = pool("ps_tr", 4, "PSUM")
    cm_psq, ps_qkv = pool("ps_qkv", 4, "PSUM")
    cm_h1, h1_pool = pool("h1", 2)

    def ln_transpose(src_rows, dstT, col):
        ht = h1_pool.tile([128, 1024], BF16, tag="h1", name="h1")
        ln_tile(src_rows, ht, ln1_triv, g1_sb, bb1_sb)
        for g in range(2):
            pst = ps_tr.tile([128, 4, 128], BF16, tag="tr", name="pst")
            for c4 in range(4):
                nc.tensor.transpose(
                    out=pst[:, c4, :],
                    in_=ht[:, (g * 4 + c4) * 128:(g * 4 + c4 + 1) * 128],
                    identity=ident_bf)
            nc.scalar.copy(
                out=dstT[:, g * 4:(g + 1) * 4, col * 128:(col + 1) * 128],
                in_=pst)

    h1TA = h1t_pool.tile([128, 8, 1024], BF16, tag="h1t", name="h1TA")
    h1TB = h1t_pool.tile([128, 8, 1024], BF16, tag="h1t", name="h1TB")

    def h1T(c, n):
        # transposed h1 slice [128, 512] for token chunk n (0..3)
        src = h1TA if n < 2 else h1TB
        return src[:, c, (n % 2) * 512:(n % 2 + 1) * 512]

    def h1Tt(c, t):
        # transposed h1 slice [128, 128] for token tile t (0..15)
        src = h1TA if t < 8 else h1TB
        return src[:, c, (t % 8) * 128:(t % 8 + 1) * 128]

    def kt_unit(n, p):
        ps = ps_qkv.tile([128, 512], F32, tag="q", name="psk")
        for c in range(8):
            nc.tensor.matmul(
                out=ps, lhsT=wkB[:, c, p * 128:(p + 1) * 128],
                rhs=h1T(c, n), start=(c == 0), stop=(c == 7))
        nc.vector.tensor_copy(out=kT[p][:, n * 512:(n + 1) * 512], in_=ps)

    def v_unit(t, n):
        ps = ps_qkv.tile([128, 512], F32, tag="q", name="psv")
        for c in range(8):
            nc.tensor.matmul(
                out=ps, lhsT=h1Tt(c, t),
                rhs=wvB[:, c, n * 512:(n + 1) * 512],
                start=(c == 0), stop=(c == 7))
        nc.vector.tensor_copy(
            out=Vt[t][:, n * 8:(n + 1) * 8, 0:64],
            in_=ps.rearrange("p (h d) -> p h d", d=64))
        if n == 1:
            nc.vector.memset(Vt[t][:, :, 64:65], 1.0)

    def q_unit(n, p):
        ps = ps_qkv.tile([128, 512], F32, tag="q", name="psq")
        for c in range(8):
            nc.tensor.matmul(
                out=ps, lhsT=h2T[:, c, p * 128:(p + 1) * 128],
                rhs=h1Th[:, c, n * 512:(n + 1) * 512],
                start=(c == 0), stop=(c == 7))
        for sh in range(2):
            s = 2 * n + sh
            nc.vector.tensor_copy(out=qTz[p][0:64, s, 0:256],
                                  in_=ps[0:64, sh * 256:(sh + 1) * 256])
            nc.vector.tensor_copy(out=qTz[p][64:128, s, 256:512],
                                  in_=ps[64:128, sh * 256:(sh + 1) * 256])

    # A half (tokens 0-1023): LN+transpose, then K(n=0,1) and V tiles 0-7
    for t in range(8):
        ln_transpose(io["x_full"][t * 128:(t + 1) * 128, :], h1TA, t)
        if t == 3:
            nc.sync.dma_start(
                out=wkB,
                in_=io["wqk"][:, 1024:2048].rearrange("(c p) n -> p c n", p=128))
        if t == 4:
            nc.sync.dma_start(
                out=wvB, in_=io["wv"].rearrange("(c p) n -> p c n", p=128))
        if t == 5:
            nc.sync.dma_start(
                out=h2T,
                in_=io["wqk"][:, 0:1024].rearrange("(c p) n -> p c n", p=128))
        if 4 <= t:
            kt_unit(0, 2 * (t - 4))
            kt_unit(0, 2 * (t - 4) + 1)
    for t in range(8):
        v_unit(t, 0)
        v_unit(t, 1)
        if t % 2 == 1:
            kt_unit(1, t - 1)
            kt_unit(1, t)
    # B half (tokens 1024-2047): transposes, then K(n=2,3)
    for t in range(8, 16):
        ln_transpose(io["x_full"][t * 128:(t + 1) * 128, :], h1TB, t - 8)
        if t == 15:
            for p in range(8):
                kt_unit(2, p)
    for p in range(8):
        kt_unit(3, p)
    # queries: LN+transpose of x_half; V tiles 8-15 run on PE while the
    # x_half tiles stream in.  wq was preloaded into h2T (same shape, not
    # written until the slot-1 proj), so q units need no weight wait.
    h1Th = h1t_pool.tile([128, 8, 1024], BF16, tag="h1t", name="h1Th")
    for th in range(8):
        ln_transpose(io["x_half"][th * 128:(th + 1) * 128, :], h1Th, th)
        v_unit(8 + th, 0)
        v_unit(8 + th, 1)
    for n in range(2):
        for p in range(8):
            q_unit(n, p)
    cm_h1.__exit__(None, None, None)
    cm_psq.__exit__(None, None, None)
    cm_pst.__exit__(None, None, None)
    cm_h1t.__exit__(None, None, None)
    cm_wqkv.__exit__(None, None, None)

    # ---------------- Phase 3: attention (sw-pipelined) ---------------------
    cm_wp, wp_pool = pool("wproj", 1)
    wpB = wp_pool.tile([128, 8, 1024], BF16, name="wpB")
    cm_att, att_pool = pool("attls", 2)

    def mask_b(m):
        # [128, 2(e), 256] view of mask tile m, broadcast over the e axis
        ap = masks_sb[:, m, :]
        return bass.AP(tensor=ap.tensor, offset=ap.offset,
                       ap=[list(ap.ap[0]), [0, 2], list(ap.ap[1])])

    cm_pt, pt_pool = pool("pt", 3)
    cm_ast, ast_pool = pool("attst", 2)
    cm_pssc, ps_sc = pool("ps_sc", 2, "PSUM")
    cm_pva_, ps_pva = pool("ps_pva", 2, "PSUM")
    cm_ppp, ps_pp = pool("ps_pp", 1, "PSUM")

    attds = [dram.tile([1024, 256], BF16, name="attd")
             for _ in range(4)]  # [c=h*64+d, 256 tq] per slot
    SCALE = HD ** -0.5

    # proj units (t, n): t-tiles 2s,2s+1 available after slot s.
    # n==1 fuses LN2 + PE-transpose of the finished x2 tile into h2T.
    def proj_unit(t, n, acts, xh):
        ps = ps_pp.tile([128, 512], F32, tag="pp", name="psp")
        for c in range(8):
            nc.tensor.matmul(
                out=ps, lhsT=acts[:, c, :],
                rhs=wpB[:, c, n * 512:(n + 1) * 512],
                start=(c == 0), stop=(c == 7))
        sl = np.s_[:, n * 512:(n + 1) * 512]
        x2t = ln_pool.tile([128, 1024], BF16, tag="x2b", name="x2t") \
            if n == 0 else proj_unit.x2t
        proj_unit.x2t = x2t
        with nc.allow_low_precision(reason="x2 residual kept in bf16"):
            nc.vector.tensor_add(out=x2t[sl], in0=ps, in1=xh[sl])
            nc.vector.tensor_add(out=x2t[sl], in0=x2t[sl], in1=bproj_sb[sl])
        if n == 1:
            nc.sync.dma_start(out=x2d[t * 128:(t + 1) * 128, :], in_=x2t)
            h2 = ln_pool.tile([128, 1024], BF16, tag="lnx", name="h2")
            ln_apply(x2t, h2, ln2_triv, g2_sb, bb2_sb)
            for g in range(2):
                pst = ps_pp.tile([128, 4, 128], BF16, tag="tr2", name="pst2")
                for c4 in range(4):
                    nc.tensor.transpose(
                        out=pst[:, c4, :],
                        in_=h2[:, (g * 4 + c4) * 128:(g * 4 + c4 + 1) * 128],
                        identity=ident_bf)
                nc.scalar.copy(
                    out=h2T[:, g * 4:(g + 1) * 4, t * 128:(t + 1) * 128],
                    in_=pst)

    def load_proj_inputs(t):
        acts = att_pool.tile([128, 8, 128], BF16, tag="attls", name="attls")
        nc.scalar.dma_start(
            out=acts,
            in_=attds[t // 2][:, (t % 2) * 128:(t % 2 + 1) * 128]
            .rearrange("(c p) n -> p c n", p=128))
        xh = ln_pool.tile([128, 1024], BF16, tag="lnx", name="xh2")
        nc.sync.dma_start(out=xh, in_=io["x_half"][t * 128:(t + 1) * 128, :])
        return acts, xh

    def attn_slot(s, proj_ts):
        ntk = 4 * (s + 1)
        npair = ntk // 2
        qc = 256 * s
        for hp in range(8):
            pva = [ps_pva.tile([128, 512], F32, tag="pv", name="pv")
                   for _ in range(2)]
            pts = {}
            for j in range(npair):
                ps = ps_sc.tile([128, 2, 512], F32, tag="sc", name="sc")
                for tk2 in range(2):
                    nc.tensor.matmul(
                        out=ps[:, tk2, :],
                        lhsT=kT[hp][:, (2 * j + tk2) * 128:
                                    (2 * j + tk2 + 1) * 128],
                        rhs=qTz[hp][:, s, :],
                        start=True, stop=True)
                pt = pt_pool.tile([128, 2, 2, 256], BF16, tag="pt", name="pt")
                pts[j] = pt
                nc.scalar.activation(
                    out=pt.rearrange("p a b c -> p (a b c)"),
                    in_=ps.rearrange("p a b -> p (a b)"),
                    func=AF.Exp, scale=SCALE)
                if j >= npair - 2:
                    for tk2 in range(2):
                        m = 4 * s + 2 * (j - (npair - 2)) + tk2
                        for e in range(2):
                            nc.vector.tensor_mul(
                                out=pt[:, tk2, e, :], in0=pt[:, tk2, e, :],
                                in1=masks_sb[:, m, :])
                if j >= 1:
                    prev = pts.pop(j - 1)
                    for tk2 in range(2):
                        for e in range(2):
                            nc.tensor.matmul(
                                out=pva[e][0:65, 0:256],
                                lhsT=Vt[2 * (j - 1) + tk2][:, 2 * hp + e, :],
                                rhs=prev[:, tk2, e, :],
                                start=(j == 1 and tk2 == 0), stop=False)
            last = pts.pop(npair - 1)
            for tk2 in range(2):
                for e in range(2):
                    nc.tensor.matmul(
                        out=pva[e][0:65, 0:256],
                        lhsT=Vt[ntk - 2 + tk2][:, 2 * hp + e, :],
                        rhs=last[:, tk2, e, :],
                        start=False, stop=(tk2 == 1))
            for e in range(2):
                rec = small.tile([1, 256], BF16, tag="rec", name="rec")
                with nc.allow_low_precision(reason="softmax denom recip bf16"):
                    nc.vector.reciprocal(out=rec, in_=pva[e][64:65, 0:256])
                bc = small.tile([64, 256], BF16, tag="bc", name="bc")
                nc.gpsimd.partition_broadcast(out_ap=bc, in_ap=rec)
                ast = ast_pool.tile([64, 256], BF16, tag="ast", name="ast")
                nc.vector.tensor_mul(out=ast, in0=pva[e][0:64, 0:256], in1=bc)
                nc.sync.dma_start(
                    out=attds[s][hp * 128 + e * 64:hp * 128 + (e + 1) * 64, :],
                    in_=ast)
            if proj_ts and hp % 4 == 3:
                t = proj_ts[hp // 4]
                acts, xh = load_proj_inputs(t)
                for n in range(2):
                    proj_unit(t, n, acts, xh)

    attn_slot(0, None)
    # w_proj load deferred past slot0 so the mask/attds DMAs win the (serial)
    # DMA engine at attention start; first use is mid-slot1
    nc.scalar.dma_start(
        out=wpB, in_=io["w_proj"].rearrange("(c p) n -> p c n", p=128))
    attn_slot(1, [0, 1])
    attn_slot(2, [2, 3])
    attn_slot(3, [4, 5])

    # proj t6..7
    for t in range(6, 8):
        acts, xh = load_proj_inputs(t)
        for n in range(2):
            proj_unit(t, n, acts, xh)

    cm_ppp.__exit__(None, None, None)
    cm_pva_.__exit__(None, None, None)
    cm_pssc.__exit__(None, None, None)
    cm_ast.__exit__(None, None, None)
    cm_pt.__exit__(None, None, None)
    cm_att.__exit__(None, None, None)
    cm_wp.__exit__(None, None, None)
    cm_masks.__exit__(None, None, None)
    cm_qt.__exit__(None, None, None)
    cm_v.__exit__(None, None, None)
    cm_kt.__exit__(None, None, None)

    # ---------------- Phase 6: FFN ------------------------------------------
    cm_ls, late_singles = pool("lsing", 1)
    b2_sb = late_singles.tile([128, 1024], F32, name="b2_sb")
    nc.gpsimd.dma_start(out=b2_sb, in_=bcast_ap(io["b2"]))
    cm_wb, wbig_pool = pool("wbig", 3)

    def load_w1b(jb):
        w1b = wbig_pool.tile([128, 8, 1024], BF16, tag="wb", name="w1b")
        for hh in range(4):
            nc.sync.dma_start(
                out=w1b[:, hh * 2:(hh + 1) * 2, :],
                in_=io["w1"][hh * 256:(hh + 1) * 256,
                             jb * 1024:(jb + 1) * 1024]
                .rearrange("(c p) n -> p c n", p=128))
        return w1b

    def load_w2b(jb):
        w2b = wbig_pool.tile([128, 8, 1024], BF16, tag="wb", name="w2b")
        for hh in range(4):
            nc.sync.dma_start(
                out=w2b[:, hh * 2:(hh + 1) * 2, :],
                in_=io["w2"][jb * 1024 + hh * 256:jb * 1024 + (hh + 1) * 256, :]
                .rearrange("(j p) n -> p j n", p=128))
        return w2b

    w1b_next = load_w1b(0)
    w2b_next = load_w2b(0)

    cm_psl, ps_late = pool("ps_late", 5, "PSUM")
    # prefetch the FFN residual (x2) tiles; oacc is initialized from them
    cm_xr, xr_pool = pool("xres", 8)
    xres = [xr_pool.tile([128, 1024], BF16, tag="xr", name="xr")
            for _ in range(8)]
    for tg in range(8):
        nc.sync.dma_start(out=xres[tg], in_=x2d[tg * 128:(tg + 1) * 128, :])

    cm_rl, relu_pool = pool("relu", 2)
    cm_oa, oacc_pool = pool("oacc", 8)
    oacc = [oacc_pool.tile([128, 1024], F32, tag="oacc", name="oacc")
            for _ in range(8)]
    for jb in range(4):
        w1b = w1b_next
        relu_b = relu_pool.tile([128, 8, 2, 512], BF16, tag="rl", name="rl")
        for pas in range(2):
            tok0 = pas * 512
            for j in range(8):
                ps = ps_late.tile([128, 512], F32, tag="l", name="psf1")
                for c in range(8):
                    nc.tensor.matmul(
                        out=ps,
                        lhsT=w1b[:, c, j * 128:(j + 1) * 128],
                        rhs=h2T[:, c, tok0:tok0 + 512],
                        start=(c == 0), stop=(c == 7))
                nc.scalar.activation(
                    out=relu_b[:, j, pas, :], in_=ps, func=AF.Relu,
                    bias=b1t_sb[:, jb * 8 + j:jb * 8 + j + 1], scale=1.0)
        w2b = w2b_next
        if jb < 3:
            w1b_next = load_w1b(jb + 1)
        for pas in range(2):
            for tl in range(4):
                tg = pas * 4 + tl
                for n in range(2):
                    ps = ps_late.tile([128, 512], F32, tag="l", name="psf2")
                    for j in range(8):
                        nc.tensor.matmul(
                            out=ps,
                            lhsT=relu_b[:, j, pas, tl * 128:(tl + 1) * 128],
                            rhs=w2b[:, j, n * 512:(n + 1) * 512],
                            start=(j == 0), stop=(j == 7))
                    sl = np.s_[:, n * 512:(n + 1) * 512]
                    if jb == 0:
                        nc.vector.tensor_add(out=oacc[tg][sl], in0=ps,
                                             in1=xres[tg][sl])
                    else:
                        nc.vector.tensor_add(out=oacc[tg][sl], in0=oacc[tg][sl],
                                             in1=ps)
                if jb == 3:
                    nc.vector.tensor_add(out=oacc[tg], in0=oacc[tg], in1=b2_sb)
                    nc.sync.dma_start(out=io["out"][tg * 128:(tg + 1) * 128, :],
                                      in_=oacc[tg])
            if jb < 3 and pas == 0:
                w2b_next = load_w2b(jb + 1)

    cm_oa.__exit__(None, None, None)
    cm_rl.__exit__(None, None, None)
    cm_xr.__exit__(None, None, None)
    cm_psl.__exit__(None, None, None)
    cm_wb.__exit__(None, None, None)
    cm_ls.__exit__(None, None, None)
    cm_h2t.__exit__(None, None, None)
    cm_dram.__exit__(None, None, None)
    cm_small.__exit__(None, None, None)
    cm_stat.__exit__(None, None, None)
    cm_ln.__exit__(None, None, None)
    cm_singles.__exit__(None, None, None)


def build(ln1_triv=True, ln2_triv=True):
    key = (ln1_triv, ln2_triv)
    if key in _CACHE:
        return _CACHE[key]
    nc = bacc.Bacc("TRN2", target_bir_lowering=False, debug=False,
                   num_devices=NCORES)
    io = {}

    def din(name, shape, dt):
        io[name] = nc.dram_tensor(name, list(shape), dt, kind="ExternalInput").ap()

    din("x_full", (2048, 1024), BF16)
    din("x_half", (1024, 1024), BF16)
    din("wqk", (1024, 2048), BF16)
    din("wv", (1024, 1024), BF16)
    din("w_proj", (1024, 1024), BF16)
    din("b_proj", (1024,), F32)
    din("w1", (1024, 4096), BF16)
    din("b1t", (128, 32), F32)
    din("w2", (4096, 1024), BF16)
    din("b2", (1024,), F32)
    din("masks", (128, 16, 256), BF16)
    if not ln1_triv:
        din("ln1_g", (1024,), F32)
        din("ln1_b", (1024,), F32)
    if not ln2_triv:
        din("ln2_g", (1024,), F32)
        din("ln2_b", (1024,), F32)
    io["out"] = nc.dram_tensor("out", [1024, 1024], F32, kind="ExternalOutput").ap()

    with tile.TileContext(nc) as tc:
        _emit_body(nc, tc, io, ln1_triv, ln2_triv)
    nc.compile()
    _CACHE[key] = (nc, io)
    return nc, io


def _chunk_starts(half):
    # slot s (0..3) holds queries [512s+256*half, 512s+256*half+256)
    return [512 * s + 256 * half for s in range(4)]


def _make_masks(half):
    """[128, 16, 256] bf16: tile m = key tile 4s+j of slot s=m//4."""
    starts = _chunk_starts(half)
    out = np.zeros((128, 16, 256), np.float32)
    tk_l = np.arange(128)[:, None]
    tq_l = np.arange(256)[None, :]
    for m in range(16):
        q0 = starts[m // 4]
        out[:, m, :] = ((m * 128 + tk_l) <= (q0 + tq_l))
    return out.astype(ml_dtypes.bfloat16)


def _prep_common(inp, ln1_triv, ln2_triv):
    wq_f = np.ascontiguousarray(inp["wq"].transpose(1, 0, 2).reshape(C, C))
    wk_f = np.ascontiguousarray(inp["wk"].transpose(1, 0, 2).reshape(C, C))
    wv_f = np.ascontiguousarray(inp["wv"].transpose(1, 0, 2).reshape(C, C))
    wqk = np.concatenate([wq_f, wk_f], axis=1).astype(ml_dtypes.bfloat16)
    b1t = np.ascontiguousarray(inp["b1"].reshape(32, 128).T).astype(np.float32)
    common = {
        "wqk": wqk,
        "wv": wv_f.astype(ml_dtypes.bfloat16),
        "w_proj": inp["w_proj"].astype(ml_dtypes.bfloat16),
        "b_proj": inp["b_proj"].astype(np.float32),
        "w1": inp["w1"].astype(ml_dtypes.bfloat16),
        "b1t": b1t,
        "w2": inp["w2"].astype(ml_dtypes.bfloat16),
        "b2": inp["b2"].astype(np.float32),
    }
    if not ln1_triv:
        common["ln1_g"] = inp["ln1_g"].astype(np.float32)
        common["ln1_b"] = inp["ln1_b"].astype(np.float32)
    if not ln2_triv:
        common["ln2_g"] = inp["ln2_g"].astype(np.float32)
        common["ln2_b"] = inp["ln2_b"].astype(np.float32)
    return common


def make_in_maps(inputs):
    inp = {k: np.asarray(v) for k, v in inputs.items()}
    x = inp["x"].astype(np.float32)
    ln1_triv = bool(np.all(inp["ln1_g"] == 1.0) and np.all(inp["ln1_b"] == 0.0))
    ln2_triv = bool(np.all(inp["ln2_g"] == 1.0) and np.all(inp["ln2_b"] == 0.0))
    common = _prep_common(inp, ln1_triv, ln2_triv)
    in_maps = []
    for i in range(NCORES):
        b, half = i // 2, i % 2
        xh = np.concatenate(
            [x[b, st:st + 256] for st in _chunk_starts(half)], axis=0)
        m = dict(common)
        m["x_full"] = np.ascontiguousarray(x[b]).astype(ml_dtypes.bfloat16)
        m["x_half"] = np.ascontiguousarray(xh).astype(ml_dtypes.bfloat16)
        m["masks"] = _make_masks(half)
        in_maps.append(m)
    return in_maps, ln1_triv, ln2_triv


def assemble(results):
    out = np.empty((B, T, C), np.float32)
    for i in range(NCORES):
        b, half = i // 2, i % 2
        o = results[i]["out"]
        for s, st in enumerate(_chunk_starts(half)):
            out[b, st:st + 256] = o[256 * s:256 * (s + 1)]
    return out


def kernel(**inputs):
    in_maps, l1, l2 = make_in_maps(inputs)
    nc, io = build(l1, l2)
    res = run_bass_kernel_spmd(nc, in_maps, list(range(NCORES)))
    return assemble(res.results)


if __name__ == "__main__":
    build()
    print("build ok")



# revision 69
# speedup vs baseline: 1.0490x; 1.0490x over previous
"""Trainium2 Bass kernel for a dense transformer block (B=4,T=2048,C=1024,H=16).

Sharding: 8 cores, zero collectives. Core i handles batch i//2; its 1024
query tokens are four 256-token chunks, one per causal "slot" s=0..3 at
tokens [512s+256*(i%2), +256).  Slot s attends to exactly 512(s+1) keys, so
the uniform SPMD program does the causally-minimal score work (40 key-tile
passes per core vs 48 for a 2x512 split); per-core host-built multiplicative
masks handle the diagonal and the core asymmetry.  All sharding on the host;
the program is identical on every core, only input data differs.

Per-core dataflow (tokens-on-free-axis for all matmul operands):
  x ships bf16.  LN1 (bn_stats) -> h1 bf16 -> PE-transpose (no DRAM bounce)
  -> h1T [C, tok] in SBUF.  QKV bf16: kT [H*64, tok] (2 heads per tile); V
  natural [tok, H*65] with a fused ones column so PV also produces the
  softmax denominator; queries go to qTz [128, slot, 512], a zero-padded
  block-diagonal layout so ONE [128,512] matmul per key tile scores both
  stacked heads (full-rate, bank-exclusive PSUM).  wq is preloaded into the
  h2T tile (same shape, dead until mid-attention).  Scores are pre-
  transposed sT[tk, tq] so softmax needs no transpose of P and no max
  subtraction; exp runs on ACT straight from PSUM with the 1/sqrt(hd) scale
  fused.  V tiles 8-15 are deferred as PE fillers inside attention slots
  0-1.  After each slot, the proj for its two 128-token tiles runs fused:
  out-proj matmul + residual (+bias) in bf16, then LN2 + PE-transpose into
  h2T immediately - no separate LN2 phase.  FFN is jb-outer (weights load
  ONCE, bf16, double-buffered), jb0 initializes the accumulator with the
  prefetched x2 residual; all matmuls bf16 with fp32 PSUM accumulation.
  Measured (cost-model timeline): ~647us/core; PE busy ~82%.
"""

import sys
import numpy as np

for _p in ("/opt/trn_rl_repo", "/root/.axon_site/_ro/trn_rl_repo"):
    if _p not in sys.path:
        sys.path.append(_p)

import ml_dtypes  # noqa: E402
import concourse.bass as bass  # noqa: E402
import concourse.bacc as bacc  # noqa: E402
import concourse.tile as tile  # noqa: E402
from concourse import mybir  # noqa: E402
from concourse.bass_utils import run_bass_kernel_spmd  # noqa: E402
from concourse.masks import make_identity  # noqa: E402

B, T, C, H, HD = 4, 2048, 1024, 16, 64
NCORES = 8
EPS = 1e-5
F32 = mybir.dt.float32
F32R = mybir.dt.float32r
BF16 = mybir.dt.bfloat16
AF = mybir.ActivationFunctionType
ALU = mybir.AluOpType

_CACHE = {}

def _emit_body(nc, tc, io, ln1_triv, ln2_triv):
    # ---------------- long-lived pools ----------------
    def pool(name, bufs, space="SBUF"):
        cm = tc.tile_pool(name=name, bufs=bufs, space=space)
        p = cm.__enter__()
        return cm, p

    cm_singles, singles = pool("singles", 1)
    cm_ln, ln_pool = pool("ln", 3)
    cm_stat, stat_pool = pool("stat", 4)
    cm_small, small = pool("small", 2)
    cm_dram, dram = pool("dram", 1, "DRAM")

    eps_t = singles.tile([128, 1], F32, name="eps")
    nc.vector.memset(eps_t, EPS)
    ident_bf = singles.tile([128, 128], BF16, name="ident_bf")
    make_identity(nc, ident_bf)
    b1t_sb = singles.tile([128, 32], F32, name="b1t_sb")
    nc.gpsimd.dma_start(out=b1t_sb, in_=io["b1t"])

    def bcast_ap(dram_ap):
        # [1024] dram vector -> [128,1024] partition-broadcast AP
        return bass.AP(
            tensor=dram_ap.tensor,
            offset=dram_ap.offset,
            ap=[[0, 128]] + list(dram_ap.ap),
        )

    bproj_sb = singles.tile([128, 1024], F32, name="bproj_sb")
    nc.gpsimd.dma_start(out=bproj_sb, in_=bcast_ap(io["b_proj"]))

    g1_sb = bb1_sb = g2_sb = bb2_sb = None
    if not ln1_triv:
        g1_sb = singles.tile([128, 1024], F32, name="g1_sb")
        nc.gpsimd.dma_start(out=g1_sb, in_=bcast_ap(io["ln1_g"]))
        bb1_sb = singles.tile([128, 1024], F32, name="bb1_sb")
        nc.gpsimd.dma_start(out=bb1_sb, in_=bcast_ap(io["ln1_b"]))
    if not ln2_triv:
        g2_sb = singles.tile([128, 1024], F32, name="g2_sb")
        nc.gpsimd.dma_start(out=g2_sb, in_=bcast_ap(io["ln2_g"]))
        bb2_sb = singles.tile([128, 1024], F32, name="bb2_sb")
        nc.gpsimd.dma_start(out=bb2_sb, in_=bcast_ap(io["ln2_b"]))

    # ---------------- LayerNorm helpers ----------------
    def ln_apply(xt, out_ap, trivial, g_sb, b_sb):
        st = stat_pool.tile([128, 2, 6], F32, tag="bnst", name="bnst")
        for sg in range(2):
            nc.vector.bn_stats(out=st[:, sg, :], in_=xt[:, sg * 512:(sg + 1) * 512])
        mv = stat_pool.tile([128, 2], F32, tag="bnmv", name="bnmv")
        nc.vector.bn_aggr(out=mv, in_=st)
        std = stat_pool.tile([128, 1], F32, tag="bnsd", name="bnsd")
        nc.scalar.activation(out=std, in_=mv[:, 1:2], func=AF.Sqrt, bias=eps_t,
                             scale=1.0)
        rstd = stat_pool.tile([128, 1], F32, tag="bnrs", name="bnrs")
        nc.vector.reciprocal(out=rstd, in_=std)
        if trivial:
            nc.vector.tensor_scalar(
                out=out_ap, in0=xt, scalar1=mv[:, 0:1], scalar2=rstd,
                op0=ALU.subtract, op1=ALU.mult)
        else:
            tmp = ln_pool.tile([128, 1024], F32, tag="lnx", name="lntmp")
            nc.vector.tensor_scalar(
                out=tmp, in0=xt, scalar1=mv[:, 0:1], scalar2=rstd,
                op0=ALU.subtract, op1=ALU.mult)
            nc.vector.tensor_mul(out=tmp, in0=tmp, in1=g_sb)
            nc.vector.tensor_add(out=out_ap, in0=tmp, in1=b_sb)

    def ln_tile(x_src_rows, out_ap, trivial, g_sb, b_sb):
        xt = ln_pool.tile([128, 1024], BF16, tag="lnx", name="lnx")
        nc.sync.dma_start(out=xt, in_=x_src_rows)
        ln_apply(xt, out_ap, trivial, g_sb, b_sb)

    # long-lived result pools, opened bottom-of-stack (LIFO discipline)
    cm_h2t, h2t_pool = pool("h2t", 1)
    h2T = h2t_pool.tile([128, 8, 1024], BF16, name="h2T")
    cm_kt, kt_pool = pool("kt", 8)
    cm_v, v_pool = pool("v", 16)
    cm_qt, qt_pool = pool("qt", 8)
    kT = [kt_pool.tile([128, 2048], BF16, tag="kt", name="kt") for _ in range(8)]
    Vt = [v_pool.tile([128, 16, 65], BF16, tag="vt", name="vt") for _ in range(16)]
    # qTz[p]: [128, 4(slot), 512] zero-padded block-diagonal queries: rows
    # 0-63 hold head-e0 q in cols 0-255, rows 64-127 head-e1 q in cols
    # 256-511.  One score matmul per key tile then covers both heads with
    # the full 128-partition contraction.
    qTz = [qt_pool.tile([128, 4, 512], BF16, tag="qt", name="qtz")
           for _ in range(8)]
    for p in range(8):
        nc.gpsimd.memset(qTz[p], 0.0)
    x2d = dram.tile([1024, 1024], BF16, name="x2d")

    cm_masks, masks_pool = pool("masks", 1)
    masks_sb = masks_pool.tile([128, 16, 256], BF16, name="masks_sb")
    nc.gpsimd.dma_start(out=masks_sb, in_=io["masks"])

    # ---------------- Phase 1+2: LN1 -> PE-transpose -> QKV -----------------
    # weight loads are emitted AFTER the first x tiles: the DMA engine is a
    # serial resource and the first LN tiles are on the critical path
    cm_wqkv, wqkv_pool = pool("wqkv", 2)
    wkB = wqkv_pool.tile([128, 8, 1024], BF16, tag="w", name="wkB")
    wvB = wqkv_pool.tile([128, 8, 1024], BF16, tag="w", name="wvB")

    cm_h1t, h1t_pool = pool("h1t", 2)
    cm_pst, ps_tr = pool("ps_tr", 4, "PSUM")
    cm_psq, ps_qkv = pool("ps_qkv", 4, "PSUM")
    cm_h1, h1_pool = pool("h1", 2)

    def ln_transpose(src_rows, dstT, col):
        ht = h1_pool.tile([128, 1024], BF16, tag="h1", name="h1")
        ln_tile(src_rows, ht, ln1_triv, g1_sb, bb1_sb)
        for g in range(2):
            pst = ps_tr.tile([128, 4, 128], BF16, tag="tr", name="pst")
            for c4 in range(4):
                nc.tensor.transpose(
                    out=pst[:, c4, :],
                    in_=ht[:, (g * 4 + c4) * 128:(g * 4 + c4 + 1) * 128],
                    identity=ident_bf)
            nc.scalar.copy(
                out=dstT[:, g * 4:(g + 1) * 4, col * 128:(col + 1) * 128],
                in_=pst)

    h1TA = h1t_pool.tile([128, 8, 1024], BF16, tag="h1t", name="h1TA")
    h1TB = h1t_pool.tile([128, 8, 1024], BF16, tag="h1t", name="h1TB")

    def h1T(c, n):
        # transposed h1 slice [128, 512] for token chunk n (0..3)
        src = h1TA if n < 2 else h1TB
        return src[:, c, (n % 2) * 512:(n % 2 + 1) * 512]

    def h1Tt(c, t):
        # transposed h1 slice [128, 128] for token tile t (0..15)
        src = h1TA if t < 8 else h1TB
        return src[:, c, (t % 8) * 128:(t % 8 + 1) * 128]

    def kt_unit(n, p):
        ps = ps_qkv.tile([128, 512], F32, tag="q", name="psk")
        for c in range(8):
            nc.tensor.matmul(
                out=ps, lhsT=wkB[:, c, p * 128:(p + 1) * 128],
                rhs=h1T(c, n), start=(c == 0), stop=(c == 7))
        nc.vector.tensor_copy(out=kT[p][:, n * 512:(n + 1) * 512], in_=ps)

    def v_unit(t, n):
        ps = ps_qkv.tile([128, 512], F32, tag="q", name="psv")
        for c in range(8):
            nc.tensor.matmul(
                out=ps, lhsT=h1Tt(c, t),
                rhs=wvB[:, c, n * 512:(n + 1) * 512],
                start=(c == 0), stop=(c == 7))
        nc.vector.tensor_copy(
            out=Vt[t][:, n * 8:(n + 1) * 8, 0:64],
            in_=ps.rearrange("p (h d) -> p h d", d=64))
        if n == 1:
            nc.vector.memset(Vt[t][:, :, 64:65], 1.0)

    def q_unit(n, p):
        ps = ps_qkv.tile([128, 512], F32, tag="q", name="psq")
        for c in range(8):
            nc.tensor.matmul(
                out=ps, lhsT=h2T[:, c, p * 128:(p + 1) * 128],
                rhs=h1Th[:, c, n * 512:(n + 1) * 512],
                start=(c == 0), stop=(c == 7))
        for sh in range(2):
            s = 2 * n + sh
            nc.vector.tensor_copy(out=qTz[p][0:64, s, 0:256],
                                  in_=ps[0:64, sh * 256:(sh + 1) * 256])
            nc.vector.tensor_copy(out=qTz[p][64:128, s, 256:512],
                                  in_=ps[64:128, sh * 256:(sh + 1) * 256])

    # A half (tokens 0-1023): LN+transpose, then K(n=0,1) and V tiles 0-7
    for t in range(8):
        ln_transpose(io["x_full"][t * 128:(t + 1) * 128, :], h1TA, t)
        if t == 3:
            nc.sync.dma_start(
                out=wkB,
                in_=io["wqk"][:, 1024:2048].rearrange("(c p) n -> p c n", p=128))
        if t == 4:
            nc.sync.dma_start(
                out=wvB, in_=io["wv"].rearrange("(c p) n -> p c n", p=128))
        if t == 5:
            nc.sync.dma_start(
                out=h2T,
                in_=io["wqk"][:, 0:1024].rearrange("(c p) n -> p c n", p=128))
        if 4 <= t:
            kt_unit(0, 2 * (t - 4))
            kt_unit(0, 2 * (t - 4) + 1)
    for t in range(8):
        v_unit(t, 0)
        v_unit(t, 1)
        if t % 2 == 1:
            kt_unit(1, t - 1)
            kt_unit(1, t)
    # B half (tokens 1024-2047): transposes, then K(n=2,3)
    for t in range(8, 16):
        ln_transpose(io["x_full"][t * 128:(t + 1) * 128, :], h1TB, t - 8)
        if t == 15:
            for p in range(8):
                kt_unit(2, p)
    for p in range(8):
        kt_unit(3, p)
    # queries: LN+transpose of x_half; V tiles 8-15 run on PE while the
    # x_half tiles stream in.  wq was preloaded into h2T (same shape, not
    # written until the slot-1 proj), so q units need no weight wait.
    h1Th = h1t_pool.tile([128, 8, 1024], BF16, tag="h1t", name="h1Th")
    for th in range(8):
        ln_transpose(io["x_half"][th * 128:(th + 1) * 128, :], h1Th, th)
        v_unit(8 + th, 0)
        v_unit(8 + th, 1)
    for n in range(2):
        for p in range(8):
            q_unit(n, p)
    cm_h1.__exit__(None, None, None)
    cm_psq.__exit__(None, None, None)
    cm_pst.__exit__(None, None, None)
    cm_h1t.__exit__(None, None, None)
    cm_wqkv.__exit__(None, None, None)

    # ---------------- Phase 3: attention (sw-pipelined) ---------------------
    cm_wp, wp_pool = pool("wproj", 1)
    wpB = wp_pool.tile([128, 8, 1024], BF16, name="wpB")
    cm_att, att_pool = pool("attls", 2)

    def mask_b(m):
        # [128, 2(e), 256] view of mask tile m, broadcast over the e axis
        ap = masks_sb[:, m, :]
        return bass.AP(tensor=ap.tensor, offset=ap.offset,
                       ap=[list(ap.ap[0]), [0, 2], list(ap.ap[1])])

    cm_pt, pt_pool = pool("pt", 3)
    cm_ast, ast_pool = pool("attst", 2)
    cm_pvs, pvs_pool = pool("pvs", 4)
    cm_pssc, ps_sc = pool("ps_sc", 2, "PSUM")
    cm_pva_, ps_pva = pool("ps_pva", 2, "PSUM")
    cm_ppp, ps_pp = pool("ps_pp", 1, "PSUM")

    attds = [dram.tile([1024, 256], BF16, name="attd")
             for _ in range(4)]  # [c=h*64+d, 256 tq] per slot
    SCALE = HD ** -0.5

    # proj units (t, n): t-tiles 2s,2s+1 available after slot s.
    # n==1 fuses LN2 + PE-transpose of the finished x2 tile into h2T.
    def proj_unit(t, n, acts, xh):
        ps = ps_pp.tile([128, 512], F32, tag="pp", name="psp")
        for c in range(8):
            nc.tensor.matmul(
                out=ps, lhsT=acts[:, c, :],
                rhs=wpB[:, c, n * 512:(n + 1) * 512],
                start=(c == 0), stop=(c == 7))
        sl = np.s_[:, n * 512:(n + 1) * 512]
        x2t = ln_pool.tile([128, 1024], BF16, tag="x2b", name="x2t") \
            if n == 0 else proj_unit.x2t
        proj_unit.x2t = x2t
        with nc.allow_low_precision(reason="x2 residual kept in bf16"):
            nc.vector.tensor_add(out=x2t[sl], in0=ps, in1=xh[sl])
            nc.vector.tensor_add(out=x2t[sl], in0=x2t[sl], in1=bproj_sb[sl])
        if n == 1:
            nc.sync.dma_start(out=x2d[t * 128:(t + 1) * 128, :], in_=x2t)
            h2 = ln_pool.tile([128, 1024], BF16, tag="lnx", name="h2")
            ln_apply(x2t, h2, ln2_triv, g2_sb, bb2_sb)
            for g in range(2):
                pst = ps_pp.tile([128, 4, 128], BF16, tag="tr2", name="pst2")
                for c4 in range(4):
                    nc.tensor.transpose(
                        out=pst[:, c4, :],
                        in_=h2[:, (g * 4 + c4) * 128:(g * 4 + c4 + 1) * 128],
                        identity=ident_bf)
                nc.scalar.copy(
                    out=h2T[:, g * 4:(g + 1) * 4, t * 128:(t + 1) * 128],
                    in_=pst)

    def load_proj_inputs(t):
        acts = att_pool.tile([128, 8, 128], BF16, tag="attls", name="attls")
        nc.scalar.dma_start(
            out=acts,
            in_=attds[t // 2][:, (t % 2) * 128:(t % 2 + 1) * 128]
            .rearrange("(c p) n -> p c n", p=128))
        xh = ln_pool.tile([128, 1024], BF16, tag="lnx", name="xh2")
        nc.sync.dma_start(out=xh, in_=io["x_half"][t * 128:(t + 1) * 128, :])
        return acts, xh

    def attn_slot(s, proj_ts):
        ntk = 4 * (s + 1)
        npair = ntk // 2
        qc = 256 * s
        for hp in range(8):
            pva = [ps_pva.tile([128, 512], F32, tag="pv", name="pv")
                   for _ in range(2)]
            pts = {}
            for j in range(npair):
                ps = ps_sc.tile([128, 2, 512], F32, tag="sc", name="sc")
                for tk2 in range(2):
                    nc.tensor.matmul(
                        out=ps[:, tk2, :],
                        lhsT=kT[hp][:, (2 * j + tk2) * 128:
                                    (2 * j + tk2 + 1) * 128],
                        rhs=qTz[hp][:, s, :],
                        start=True, stop=True)
                pt = pt_pool.tile([128, 2, 2, 256], BF16, tag="pt", name="pt")
                pts[j] = pt
                nc.scalar.activation(
                    out=pt.rearrange("p a b c -> p (a b c)"),
                    in_=ps.rearrange("p a b -> p (a b)"),
                    func=AF.Exp, scale=SCALE)
                if j >= npair - 2:
                    for tk2 in range(2):
                        m = 4 * s + 2 * (j - (npair - 2)) + tk2
                        for e in range(2):
                            nc.vector.tensor_mul(
                                out=pt[:, tk2, e, :], in0=pt[:, tk2, e, :],
                                in1=masks_sb[:, m, :])
                if j >= 1:
                    prev = pts.pop(j - 1)
                    for tk2 in range(2):
                        for e in range(2):
                            nc.tensor.matmul(
                                out=pva[e][0:65, 0:256],
                                lhsT=Vt[2 * (j - 1) + tk2][:, 2 * hp + e, :],
                                rhs=prev[:, tk2, e, :],
                                start=(j == 1 and tk2 == 0), stop=False)
            last = pts.pop(npair - 1)
            for tk2 in range(2):
                for e in range(2):
                    nc.tensor.matmul(
                        out=pva[e][0:65, 0:256],
                        lhsT=Vt[ntk - 2 + tk2][:, 2 * hp + e, :],
                        rhs=last[:, tk2, e, :],
                        start=False, stop=(tk2 == 1))
            for e in range(2):
                # one fast copy frees the pva psum bank for the next head
                # pair; the recip/broadcast/normalize chain runs off-path
                pvs = pvs_pool.tile([65, 256], BF16, tag="pvs", name="pvs")
                with nc.allow_low_precision(reason="attn out normalized bf16"):
                    nc.vector.tensor_copy(out=pvs, in_=pva[e][0:65, 0:256])
                rec = small.tile([1, 256], BF16, tag="rec", name="rec")
                with nc.allow_low_precision(reason="softmax denom recip bf16"):
                    nc.vector.reciprocal(out=rec, in_=pvs[64:65, :])
                bc = small.tile([64, 256], BF16, tag="bc", name="bc")
                nc.gpsimd.partition_broadcast(out_ap=bc, in_ap=rec)
                ast = ast_pool.tile([64, 256], BF16, tag="ast", name="ast")
                nc.vector.tensor_mul(out=ast, in0=pvs[0:64, :], in1=bc)
                nc.sync.dma_start(
                    out=attds[s][hp * 128 + e * 64:hp * 128 + (e + 1) * 64, :],
                    in_=ast)
            if proj_ts and hp % 4 == 3:
                t = proj_ts[hp // 4]
                acts, xh = load_proj_inputs(t)
                for n in range(2):
                    proj_unit(t, n, acts, xh)

    attn_slot(0, None)
    # w_proj load deferred past slot0 so the mask/attds DMAs win the (serial)
    # DMA engine at attention start; first use is mid-slot1
    nc.scalar.dma_start(
        out=wpB, in_=io["w_proj"].rearrange("(c p) n -> p c n", p=128))
    attn_slot(1, [0, 1])
    attn_slot(2, [2, 3])
    attn_slot(3, [4, 5])

    # proj t6..7
    for t in range(6, 8):
        acts, xh = load_proj_inputs(t)
        for n in range(2):
            proj_unit(t, n, acts, xh)

    cm_ppp.__exit__(None, None, None)
    cm_pva_.__exit__(None, None, None)
    cm_pssc.__exit__(None, None, None)
    cm_pvs.__exit__(None, None, None)
    cm_ast.__exit__(None, None, None)
    cm_pt.__exit__(None, None, None)
    cm_att.__exit__(None, None, None)
    cm_wp.__exit__(None, None, None)
    cm_masks.__exit__(None, None, None)
    cm_qt.__exit__(None, None, None)
    cm_v.__exit__(None, None, None)
    cm_kt.__exit__(None, None, None)

    # ---------------- Phase 6: FFN ------------------------------------------
    cm_ls, late_singles = pool("lsing", 1)
    b2_sb = late_singles.tile([128, 1024], F32, name="b2_sb")
    nc.gpsimd.dma_start(out=b2_sb, in_=bcast_ap(io["b2"]))
    cm_wb, wbig_pool = pool("wbig", 3)

    def load_w1b(jb):
        w1b = wbig_pool.tile([128, 8, 1024], BF16, tag="wb", name="w1b")
        for hh in range(4):
            nc.sync.dma_start(
                out=w1b[:, hh * 2:(hh + 1) * 2, :],
                in_=io["w1"][hh * 256:(hh + 1) * 256,
                             jb * 1024:(jb + 1) * 1024]
                .rearrange("(c p) n -> p c n", p=128))
        return w1b

    def load_w2b(jb):
        w2b = wbig_pool.tile([128, 8, 1024], BF16, tag="wb", name="w2b")
        for hh in range(4):
            nc.sync.dma_start(
                out=w2b[:, hh * 2:(hh + 1) * 2, :],
                in_=io["w2"][jb * 1024 + hh * 256:jb * 1024 + (hh + 1) * 256, :]
                .rearrange("(j p) n -> p j n", p=128))
        return w2b

    w1b_next = load_w1b(0)
    w2b_next = load_w2b(0)

    cm_psl, ps_late = pool("ps_late", 5, "PSUM")
    # prefetch the FFN residual (x2) tiles; oacc is initialized from them
    cm_xr, xr_pool = pool("xres", 8)
    xres = [xr_pool.tile([128, 1024], BF16, tag="xr", name="xr")
            for _ in range(8)]
    for tg in range(8):
        nc.sync.dma_start(out=xres[tg], in_=x2d[tg * 128:(tg + 1) * 128, :])

    cm_rl, relu_pool = pool("relu", 2)
    cm_oa, oacc_pool = pool("oacc", 8)
    oacc = [oacc_pool.tile([128, 1024], F32, tag="oacc", name="oacc")
            for _ in range(8)]
    for jb in range(4):
        w1b = w1b_next
        relu_b = relu_pool.tile([128, 8, 2, 512], BF16, tag="rl", name="rl")
        for pas in range(2):
            tok0 = pas * 512
            for j in range(8):
                ps = ps_late.tile([128, 512], F32, tag="l", name="psf1")
                for c in range(8):
                    nc.tensor.matmul(
                        out=ps,
                        lhsT=w1b[:, c, j * 128:(j + 1) * 128],
                        rhs=h2T[:, c, tok0:tok0 + 512],
                        start=(c == 0), stop=(c == 7))
                nc.scalar.activation(
                    out=relu_b[:, j, pas, :], in_=ps, func=AF.Relu,
                    bias=b1t_sb[:, jb * 8 + j:jb * 8 + j + 1], scale=1.0)
        w2b = w2b_next
        if jb < 3:
            w1b_next = load_w1b(jb + 1)
        for pas in range(2):
            for tl in range(4):
                tg = pas * 4 + tl
                for n in range(2):
                    ps = ps_late.tile([128, 512], F32, tag="l", name="psf2")
                    for j in range(8):
                        nc.tensor.matmul(
                            out=ps,
                            lhsT=relu_b[:, j, pas, tl * 128:(tl + 1) * 128],
                            rhs=w2b[:, j, n * 512:(n + 1) * 512],
                            start=(j == 0), stop=(j == 7))
                    sl = np.s_[:, n * 512:(n + 1) * 512]
                    if jb == 0:
                        nc.vector.tensor_add(out=oacc[tg][sl], in0=ps,
                                             in1=xres[tg][sl])
                    else:
                        nc.vector.tensor_add(out=oacc[tg][sl], in0=oacc[tg][sl],
                                             in1=ps)
                if jb == 3:
                    nc.vector.tensor_add(out=oacc[tg], in0=oacc[tg], in1=b2_sb)
                    nc.sync.dma_start(out=io["out"][tg * 128:(tg + 1) * 128, :],
                                      in_=oacc[tg])
            if jb < 3 and pas == 0:
                w2b_next = load_w2b(jb + 1)

    cm_oa.__exit__(None, None, None)
    cm_rl.__exit__(None, None, None)
    cm_xr.__exit__(None, None, None)
    cm_psl.__exit__(None, None, None)
    cm_wb.__exit__(None, None, None)
    cm_ls.__exit__(None, None, None)
    cm_h2t.__exit__(None, None, None)
    cm_dram.__exit__(None, None, None)
    cm_small.__exit__(None, None, None)
    cm_stat.__exit__(None, None, None)
    cm_ln.__exit__(None, None, None)
    cm_singles.__exit__(None, None, None)


def build(ln1_triv=True, ln2_triv=True):
    key = (ln1_triv, ln2_triv)
    if key in _CACHE:
        return _CACHE[key]
    nc = bacc.Bacc("TRN2", target_bir_lowering=False, debug=False,
                   num_devices=NCORES)
    io = {}

    def din(name, shape, dt):
        io[name] = nc.dram_tensor(name, list(shape), dt, kind="ExternalInput").ap()

    din("x_full", (2048, 1024), BF16)
    din("x_half", (1024, 1024), BF16)
    din("wqk", (1024, 2048), BF16)
    din("wv", (1024, 1024), BF16)
    din("w_proj", (1024, 1024), BF16)
    din("b_proj", (1024,), F32)
    din("w1", (1024, 4096), BF16)
    din("b1t", (128, 32), F32)
    din("w2", (4096, 1024), BF16)
    din("b2", (1024,), F32)
    din("masks", (128, 16, 256), BF16)
    if not ln1_triv:
        din("ln1_g", (1024,), F32)
        din("ln1_b", (1024,), F32)
    if not ln2_triv:
        din("ln2_g", (1024,), F32)
        din("ln2_b", (1024,), F32)
    io["out"] = nc.dram_tensor("out", [1024, 1024], F32, kind="ExternalOutput").ap()

    with tile.TileContext(nc) as tc:
        _emit_body(nc, tc, io, ln1_triv, ln2_triv)
    nc.compile()
    _CACHE[key] = (nc, io)
    return nc, io


def _chunk_starts(half):
    # slot s (0..3) holds queries [512s+256*half, 512s+256*half+256)
    return [512 * s + 256 * half for s in range(4)]


def _make_masks(half):
    """[128, 16, 256] bf16: tile m = key tile 4s+j of slot s=m//4."""
    starts = _chunk_starts(half)
    out = np.zeros((128, 16, 256), np.float32)
    tk_l = np.arange(128)[:, None]
    tq_l = np.arange(256)[None, :]
    for m in range(16):
        q0 = starts[m // 4]
        out[:, m, :] = ((m * 128 + tk_l) <= (q0 + tq_l))
    return out.astype(ml_dtypes.bfloat16)


def _prep_common(inp, ln1_triv, ln2_triv):
    wq_f = np.ascontiguousarray(inp["wq"].transpose(1, 0, 2).reshape(C, C))
    wk_f = np.ascontiguousarray(inp["wk"].transpose(1, 0, 2).reshape(C, C))
    wv_f = np.ascontiguousarray(inp["wv"].transpose(1, 0, 2).reshape(C, C))
    wqk = np.concatenate([wq_f, wk_f], axis=1).astype(ml_dtypes.bfloat16)
    b1t = np.ascontiguousarray(inp["b1"].reshape(32, 128).T).astype(np.float32)
    common = {
        "wqk": wqk,
        "wv": wv_f.astype(ml_dtypes.bfloat16),
        "w_proj": inp["w_proj"].astype(ml_dtypes.bfloat16),
        "b_proj": inp["b_proj"].astype(np.float32),
        "w1": inp["w1"].astype(ml_dtypes.bfloat16),
        "b1t": b1t,
        "w2": inp["w2"].astype(ml_dtypes.bfloat16),
        "b2": inp["b2"].astype(np.float32),
    }
    if not ln1_triv:
        common["ln1_g"] = inp["ln1_g"].astype(np.float32)
        common["ln1_b"] = inp["ln1_b"].astype(np.float32)
    if not ln2_triv:
        common["ln2_g"] = inp["ln2_g"].astype(np.float32)
        common["ln2_b"] = inp["ln2_b"].astype(np.float32)
    return common


def make_in_maps(inputs):
    inp = {k: np.asarray(v) for k, v in inputs.items()}
    x = inp["x"].astype(np.float32)
    ln1_triv = bool(np.all(inp["ln1_g"] == 1.0) and np.all(inp["ln1_b"] == 0.0))
    ln2_triv = bool(np.all(inp["ln2_g"] == 1.0) and np.all(inp["ln2_b"] == 0.0))
    common = _prep_common(inp, ln1_triv, ln2_triv)
    in_maps = []
    for i in range(NCORES):
        b, half = i // 2, i % 2
        xh = np.concatenate(
            [x[b, st:st + 256] for st in _chunk_starts(half)], axis=0)
        m = dict(common)
        m["x_full"] = np.ascontiguousarray(x[b]).astype(ml_dtypes.bfloat16)
        m["x_half"] = np.ascontiguousarray(xh).astype(ml_dtypes.bfloat16)
        m["masks"] = _make_masks(half)
        in_maps.append(m)
    return in_maps, ln1_triv, ln2_triv


def assemble(results):
    out = np.empty((B, T, C), np.float32)
    for i in range(NCORES):
        b, half = i // 2, i % 2
        o = results[i]["out"]
        for s, st in enumerate(_chunk_starts(half)):
            out[b, st:st + 256] = o[256 * s:256 * (s + 1)]
    return out


def kernel(**inputs):
    in_maps, l1, l2 = make_in_maps(inputs)
    nc, io = build(l1, l2)
    res = run_bass_kernel_spmd(nc, in_maps, list(range(NCORES)))
    return assemble(res.results)


if __name__ == "__main__":
    build()
    print("build ok")



# revision 70
# speedup vs baseline: 1.1203x; 1.0679x over previous
"""Trainium2 Bass kernel for a dense transformer block (B=4,T=2048,C=1024,H=16).

Sharding: 8 cores, zero collectives. Core i handles batch i//2; its 1024
query tokens are four 256-token chunks, one per causal "slot" s=0..3 at
tokens [512s+256*(i%2), +256).  Slot s attends to exactly 512(s+1) keys, so
the uniform SPMD program does the causally-minimal score work (40 key-tile
passes per core vs 48 for a 2x512 split); per-core host-built multiplicative
masks handle the diagonal and the core asymmetry.  All sharding on the host;
the program is identical on every core, only input data differs.

Per-core dataflow (tokens-on-free-axis for all matmul operands):
  x ships bf16.  LN1 (bn_stats) -> h1 bf16 -> PE-transpose (no DRAM bounce)
  -> h1T [C, tok] in SBUF.  QKV bf16: kT [H*64, tok] (2 heads per tile); V
  natural [tok, H*65] with a fused ones column so PV also produces the
  softmax denominator; queries go to qTz [128, slot, 512], a zero-padded
  block-diagonal layout so ONE [128,512] matmul per key tile scores both
  stacked heads (full-rate, bank-exclusive PSUM).  wq is preloaded into the
  h2T tile (same shape, dead until mid-attention).  Scores are pre-
  transposed sT[tk, tq] so softmax needs no transpose of P and no max
  subtraction; exp runs on ACT straight from PSUM with the 1/sqrt(hd) scale
  fused.  V tiles 8-15 are deferred as PE fillers inside attention slots
  0-1.  After each slot, the proj for its two 128-token tiles runs fused:
  out-proj matmul + residual (+bias) in bf16, then LN2 + PE-transpose into
  h2T immediately - no separate LN2 phase.  FFN is jb-outer (weights load
  ONCE, bf16, double-buffered), jb0 initializes the accumulator with the
  prefetched x2 residual; all matmuls bf16 with fp32 PSUM accumulation.
  After each head-pair a single DVE copy moves the attention
  accumulator out of PSUM so the recip/broadcast/normalize chain runs
  off the PE critical path.  Measured (cost-model timeline): ~636us/core.
"""

import sys
import numpy as np

for _p in ("/opt/trn_rl_repo", "/root/.axon_site/_ro/trn_rl_repo"):
    if _p not in sys.path:
        sys.path.append(_p)

import ml_dtypes  # noqa: E402
import concourse.bass as bass  # noqa: E402
import concourse.bacc as bacc  # noqa: E402
import concourse.tile as tile  # noqa: E402
from concourse import mybir  # noqa: E402
from concourse.bass_utils import run_bass_kernel_spmd  # noqa: E402
from concourse.masks import make_identity  # noqa: E402

B, T, C, H, HD = 4, 2048, 1024, 16, 64
NCORES = 8
EPS = 1e-5
F32 = mybir.dt.float32
F32R = mybir.dt.float32r
BF16 = mybir.dt.bfloat16
AF = mybir.ActivationFunctionType
ALU = mybir.AluOpType

_CACHE = {}

def _emit_body(nc, tc, io, ln1_triv, ln2_triv):
    # ---------------- long-lived pools ----------------
    def pool(name, bufs, space="SBUF"):
        cm = tc.tile_pool(name=name, bufs=bufs, space=space)
        p = cm.__enter__()
        return cm, p

    cm_singles, singles = pool("singles", 1)
    cm_ln, ln_pool = pool("ln", 3)
    cm_stat, stat_pool = pool("stat", 4)
    cm_small, small = pool("small", 2)
    cm_dram, dram = pool("dram", 1, "DRAM")

    eps_t = singles.tile([128, 1], F32, name="eps")
    nc.vector.memset(eps_t, EPS)
    ident_bf = singles.tile([128, 128], BF16, name="ident_bf")
    make_identity(nc, ident_bf)
    b1t_sb = singles.tile([128, 32], F32, name="b1t_sb")
    nc.gpsimd.dma_start(out=b1t_sb, in_=io["b1t"])

    def bcast_ap(dram_ap):
        # [1024] dram vector -> [128,1024] partition-broadcast AP
        return bass.AP(
            tensor=dram_ap.tensor,
            offset=dram_ap.offset,
            ap=[[0, 128]] + list(dram_ap.ap),
        )

    bproj_sb = singles.tile([128, 1024], F32, name="bproj_sb")
    nc.gpsimd.dma_start(out=bproj_sb, in_=bcast_ap(io["b_proj"]))

    g1_sb = bb1_sb = g2_sb = bb2_sb = None
    if not ln1_triv:
        g1_sb = singles.tile([128, 1024], F32, name="g1_sb")
        nc.gpsimd.dma_start(out=g1_sb, in_=bcast_ap(io["ln1_g"]))
        bb1_sb = singles.tile([128, 1024], F32, name="bb1_sb")
        nc.gpsimd.dma_start(out=bb1_sb, in_=bcast_ap(io["ln1_b"]))
    if not ln2_triv:
        g2_sb = singles.tile([128, 1024], F32, name="g2_sb")
        nc.gpsimd.dma_start(out=g2_sb, in_=bcast_ap(io["ln2_g"]))
        bb2_sb = singles.tile([128, 1024], F32, name="bb2_sb")
        nc.gpsimd.dma_start(out=bb2_sb, in_=bcast_ap(io["ln2_b"]))

    # ---------------- LayerNorm helpers ----------------
    def ln_apply(xt, out_ap, trivial, g_sb, b_sb):
        st = stat_pool.tile([128, 2, 6], F32, tag="bnst", name="bnst")
        for sg in range(2):
            nc.vector.bn_stats(out=st[:, sg, :], in_=xt[:, sg * 512:(sg + 1) * 512])
        mv = stat_pool.tile([128, 2], F32, tag="bnmv", name="bnmv")
        nc.vector.bn_aggr(out=mv, in_=st)
        std = stat_pool.tile([128, 1], F32, tag="bnsd", name="bnsd")
        nc.scalar.activation(out=std, in_=mv[:, 1:2], func=AF.Sqrt, bias=eps_t,
                             scale=1.0)
        rstd = stat_pool.tile([128, 1], F32, tag="bnrs", name="bnrs")
        nc.vector.reciprocal(out=rstd, in_=std)
        if trivial:
            nc.vector.tensor_scalar(
                out=out_ap, in0=xt, scalar1=mv[:, 0:1], scalar2=rstd,
                op0=ALU.subtract, op1=ALU.mult)
        else:
            tmp = ln_pool.tile([128, 1024], F32, tag="lnx", name="lntmp")
            nc.vector.tensor_scalar(
                out=tmp, in0=xt, scalar1=mv[:, 0:1], scalar2=rstd,
                op0=ALU.subtract, op1=ALU.mult)
            nc.vector.tensor_mul(out=tmp, in0=tmp, in1=g_sb)
            nc.vector.tensor_add(out=out_ap, in0=tmp, in1=b_sb)

    def ln_tile(x_src_rows, out_ap, trivial, g_sb, b_sb):
        xt = ln_pool.tile([128, 1024], BF16, tag="lnx", name="lnx")
        nc.sync.dma_start(out=xt, in_=x_src_rows)
        ln_apply(xt, out_ap, trivial, g_sb, b_sb)

    # long-lived result pools, opened bottom-of-stack (LIFO discipline)
    cm_h2t, h2t_pool = pool("h2t", 1)
    h2T = h2t_pool.tile([128, 8, 1024], BF16, name="h2T")
    cm_kt, kt_pool = pool("kt", 8)
    cm_v, v_pool = pool("v", 16)
    cm_qt, qt_pool = pool("qt", 8)
    kT = [kt_pool.tile([128, 2048], BF16, tag="kt", name="kt") for _ in range(8)]
    Vt = [v_pool.tile([128, 16, 65], BF16, tag="vt", name="vt") for _ in range(16)]
    # qTz[p]: [128, 4(slot), 512] zero-padded block-diagonal queries: rows
    # 0-63 hold head-e0 q in cols 0-255, rows 64-127 head-e1 q in cols
    # 256-511.  One score matmul per key tile then covers both heads with
    # the full 128-partition contraction.
    qTz = [qt_pool.tile([128, 4, 512], BF16, tag="qt", name="qtz")
           for _ in range(8)]
    for p in range(8):
        nc.gpsimd.memset(qTz[p], 0.0)
    x2d = dram.tile([1024, 1024], BF16, name="x2d")

    cm_masks, masks_pool = pool("masks", 1)
    masks_sb = masks_pool.tile([128, 16, 256], BF16, name="masks_sb")
    nc.gpsimd.dma_start(out=masks_sb, in_=io["masks"])

    # ---------------- Phase 1+2: LN1 -> PE-transpose -> QKV -----------------
    # weight loads are emitted AFTER the first x tiles: the DMA engine is a
    # serial resource and the first LN tiles are on the critical path
    cm_wqkv, wqkv_pool = pool("wqkv", 2)
    wkB = wqkv_pool.tile([128, 8, 1024], BF16, tag="w", name="wkB")
    wvB = wqkv_pool.tile([128, 8, 1024], BF16, tag="w", name="wvB")

    cm_h1t, h1t_pool = pool("h1t", 2)
    cm_pst, ps_tr = pool("ps_tr", 4, "PSUM")
    cm_psq, ps_qkv = pool("ps_qkv", 4, "PSUM")
    cm_h1, h1_pool = pool("h1", 2)

    def ln_transpose(src_rows, dstT, col):
        ht = h1_pool.tile([128, 1024], BF16, tag="h1", name="h1")
        ln_tile(src_rows, ht, ln1_triv, g1_sb, bb1_sb)
        for g in range(2):
            pst = ps_tr.tile([128, 4, 128], BF16, tag="tr", name="pst")
            for c4 in range(4):
                nc.tensor.transpose(
                    out=pst[:, c4, :],
                    in_=ht[:, (g * 4 + c4) * 128:(g * 4 + c4 + 1) * 128],
                    identity=ident_bf)
            nc.scalar.copy(
                out=dstT[:, g * 4:(g + 1) * 4, col * 128:(col + 1) * 128],
                in_=pst)

    h1TA = h1t_pool.tile([128, 8, 1024], BF16, tag="h1t", name="h1TA")
    h1TB = h1t_pool.tile([128, 8, 1024], BF16, tag="h1t", name="h1TB")

    def h1T(c, n):
        # transposed h1 slice [128, 512] for token chunk n (0..3)
        src = h1TA if n < 2 else h1TB
        return src[:, c, (n % 2) * 512:(n % 2 + 1) * 512]

    def h1Tt(c, t):
        # transposed h1 slice [128, 128] for token tile t (0..15)
        src = h1TA if t < 8 else h1TB
        return src[:, c, (t % 8) * 128:(t % 8 + 1) * 128]

    def kt_unit(n, p):
        ps = ps_qkv.tile([128, 512], F32, tag="q", name="psk")
        for c in range(8):
            nc.tensor.matmul(
                out=ps, lhsT=wkB[:, c, p * 128:(p + 1) * 128],
                rhs=h1T(c, n), start=(c == 0), stop=(c == 7))
        nc.vector.tensor_copy(out=kT[p][:, n * 512:(n + 1) * 512], in_=ps)

    def v_unit(t, n):
        ps = ps_qkv.tile([128, 512], F32, tag="q", name="psv")
        for c in range(8):
            nc.tensor.matmul(
                out=ps, lhsT=h1Tt(c, t),
                rhs=wvB[:, c, n * 512:(n + 1) * 512],
                start=(c == 0), stop=(c == 7))
        nc.vector.tensor_copy(
            out=Vt[t][:, n * 8:(n + 1) * 8, 0:64],
            in_=ps.rearrange("p (h d) -> p h d", d=64))
        if n == 1:
            nc.vector.memset(Vt[t][:, :, 64:65], 1.0)

    def q_unit(n, p):
        ps = ps_qkv.tile([128, 512], F32, tag="q", name="psq")
        for c in range(8):
            nc.tensor.matmul(
                out=ps, lhsT=h2T[:, c, p * 128:(p + 1) * 128],
                rhs=h1Th[:, c, n * 512:(n + 1) * 512],
                start=(c == 0), stop=(c == 7))
        for sh in range(2):
            s = 2 * n + sh
            nc.vector.tensor_copy(out=qTz[p][0:64, s, 0:256],
                                  in_=ps[0:64, sh * 256:(sh + 1) * 256])
            nc.vector.tensor_copy(out=qTz[p][64:128, s, 256:512],
                                  in_=ps[64:128, sh * 256:(sh + 1) * 256])

    # A half (tokens 0-1023): LN+transpose, then K(n=0,1) and V tiles 0-7
    for t in range(8):
        ln_transpose(io["x_full"][t * 128:(t + 1) * 128, :], h1TA, t)
        if t == 3:
            nc.sync.dma_start(
                out=wkB,
                in_=io["wqk"][:, 1024:2048].rearrange("(c p) n -> p c n", p=128))
        if t == 4:
            nc.sync.dma_start(
                out=wvB, in_=io["wv"].rearrange("(c p) n -> p c n", p=128))
        if t == 5:
            nc.sync.dma_start(
                out=h2T,
                in_=io["wqk"][:, 0:1024].rearrange("(c p) n -> p c n", p=128))
        if 4 <= t:
            kt_unit(0, 2 * (t - 4))
            kt_unit(0, 2 * (t - 4) + 1)
    for t in range(8):
        v_unit(t, 0)
        v_unit(t, 1)
        if t % 2 == 1:
            kt_unit(1, t - 1)
            kt_unit(1, t)
    # B half (tokens 1024-2047): transposes, then K(n=2,3)
    for t in range(8, 16):
        ln_transpose(io["x_full"][t * 128:(t + 1) * 128, :], h1TB, t - 8)
        if t == 15:
            for p in range(8):
                kt_unit(2, p)
    for p in range(8):
        kt_unit(3, p)
    # queries: LN+transpose of x_half; V tiles 8-15 run on PE while the
    # x_half tiles stream in.  wq was preloaded into h2T (same shape, not
    # written until the slot-1 proj), so q units need no weight wait.
    h1Th = h1t_pool.tile([128, 8, 1024], BF16, tag="h1t", name="h1Th")
    for th in range(8):
        ln_transpose(io["x_half"][th * 128:(th + 1) * 128, :], h1Th, th)
        v_unit(8 + th, 0)
        v_unit(8 + th, 1)
    for n in range(2):
        for p in range(8):
            q_unit(n, p)
    cm_h1.__exit__(None, None, None)
    cm_psq.__exit__(None, None, None)
    cm_pst.__exit__(None, None, None)
    cm_h1t.__exit__(None, None, None)
    cm_wqkv.__exit__(None, None, None)

    # ---------------- Phase 3: attention (sw-pipelined) ---------------------
    cm_wp, wp_pool = pool("wproj", 1)
    wpB = wp_pool.tile([128, 8, 1024], BF16, name="wpB")
    cm_att, att_pool = pool("attls", 2)

    def mask_b(m):
        # [128, 2(e), 256] view of mask tile m, broadcast over the e axis
        ap = masks_sb[:, m, :]
        return bass.AP(tensor=ap.tensor, offset=ap.offset,
                       ap=[list(ap.ap[0]), [0, 2], list(ap.ap[1])])

    cm_pt, pt_pool = pool("pt", 3)
    cm_ast, ast_pool = pool("attst", 2)
    cm_pvs, pvs_pool = pool("pvs", 4)
    cm_pssc, ps_sc = pool("ps_sc", 2, "PSUM")
    cm_pva_, ps_pva = pool("ps_pva", 2, "PSUM")
    cm_ppp, ps_pp = pool("ps_pp", 1, "PSUM")

    attds = [dram.tile([1024, 256], BF16, name="attd")
             for _ in range(4)]  # [c=h*64+d, 256 tq] per slot
    SCALE = HD ** -0.5

    # proj units (t, n): t-tiles 2s,2s+1 available after slot s.
    # n==1 fuses LN2 + PE-transpose of the finished x2 tile into h2T.
    def proj_unit(t, n, acts, xh):
        ps = ps_pp.tile([128, 512], F32, tag="pp", name="psp")
        for c in range(8):
            nc.tensor.matmul(
                out=ps, lhsT=acts[:, c, :],
                rhs=wpB[:, c, n * 512:(n + 1) * 512],
                start=(c == 0), stop=(c == 7))
        sl = np.s_[:, n * 512:(n + 1) * 512]
        x2t = ln_pool.tile([128, 1024], BF16, tag="x2b", name="x2t") \
            if n == 0 else proj_unit.x2t
        proj_unit.x2t = x2t
        with nc.allow_low_precision(reason="x2 residual kept in bf16"):
            nc.vector.tensor_add(out=x2t[sl], in0=ps, in1=xh[sl])
            nc.vector.tensor_add(out=x2t[sl], in0=x2t[sl], in1=bproj_sb[sl])
        if n == 1:
            nc.sync.dma_start(out=x2d[t * 128:(t + 1) * 128, :], in_=x2t)
            h2 = ln_pool.tile([128, 1024], BF16, tag="lnx", name="h2")
            ln_apply(x2t, h2, ln2_triv, g2_sb, bb2_sb)
            for g in range(2):
                pst = ps_pp.tile([128, 4, 128], BF16, tag="tr2", name="pst2")
                for c4 in range(4):
                    nc.tensor.transpose(
                        out=pst[:, c4, :],
                        in_=h2[:, (g * 4 + c4) * 128:(g * 4 + c4 + 1) * 128],
                        identity=ident_bf)
                nc.scalar.copy(
                    out=h2T[:, g * 4:(g + 1) * 4, t * 128:(t + 1) * 128],
                    in_=pst)

    def load_proj_inputs(t):
        acts = att_pool.tile([128, 8, 128], BF16, tag="attls", name="attls")
        nc.scalar.dma_start(
            out=acts,
            in_=attds[t // 2][:, (t % 2) * 128:(t % 2 + 1) * 128]
            .rearrange("(c p) n -> p c n", p=128))
        xh = ln_pool.tile([128, 1024], BF16, tag="lnx", name="xh2")
        nc.sync.dma_start(out=xh, in_=io["x_half"][t * 128:(t + 1) * 128, :])
        return acts, xh

    def attn_slot(s, proj_ts):
        ntk = 4 * (s + 1)
        npair = ntk // 2
        qc = 256 * s
        for hp in range(8):
            pva = [ps_pva.tile([128, 512], F32, tag="pv", name="pv")
                   for _ in range(2)]
            pts = {}
            for j in range(npair):
                ps = ps_sc.tile([128, 2, 512], F32, tag="sc", name="sc")
                for tk2 in range(2):
                    nc.tensor.matmul(
                        out=ps[:, tk2, :],
                        lhsT=kT[hp][:, (2 * j + tk2) * 128:
                                    (2 * j + tk2 + 1) * 128],
                        rhs=qTz[hp][:, s, :],
                        start=True, stop=True)
                pt = pt_pool.tile([128, 2, 2, 256], BF16, tag="pt", name="pt")
                pts[j] = pt
                nc.scalar.activation(
                    out=pt.rearrange("p a b c -> p (a b c)"),
                    in_=ps.rearrange("p a b -> p (a b)"),
                    func=AF.Exp, scale=SCALE)
                if j >= npair - 2:
                    for tk2 in range(2):
                        m = 4 * s + 2 * (j - (npair - 2)) + tk2
                        for e in range(2):
                            nc.vector.tensor_mul(
                                out=pt[:, tk2, e, :], in0=pt[:, tk2, e, :],
                                in1=masks_sb[:, m, :])
                if j >= 1:
                    prev = pts.pop(j - 1)
                    for tk2 in range(2):
                        for e in range(2):
                            nc.tensor.matmul(
                                out=pva[e][0:65, 0:256],
                                lhsT=Vt[2 * (j - 1) + tk2][:, 2 * hp + e, :],
                                rhs=prev[:, tk2, e, :],
                                start=(j == 1 and tk2 == 0), stop=False)
            last = pts.pop(npair - 1)
            for tk2 in range(2):
                for e in range(2):
                    nc.tensor.matmul(
                        out=pva[e][0:65, 0:256],
                        lhsT=Vt[ntk - 2 + tk2][:, 2 * hp + e, :],
                        rhs=last[:, tk2, e, :],
                        start=False, stop=(tk2 == 1))
            for e in range(2):
                # one fast copy frees the pva psum bank for the next head
                # pair; the recip/broadcast/normalize chain runs off-path
                pvs = pvs_pool.tile([65, 256], BF16, tag="pvs", name="pvs")
                with nc.allow_low_precision(reason="attn out normalized bf16"):
                    nc.vector.tensor_copy(out=pvs, in_=pva[e][0:65, 0:256])
                rec = small.tile([1, 256], BF16, tag="rec", name="rec")
                with nc.allow_low_precision(reason="softmax denom recip bf16"):
                    nc.vector.reciprocal(out=rec, in_=pvs[64:65, :])
                bc = small.tile([64, 256], BF16, tag="bc", name="bc")
                nc.gpsimd.partition_broadcast(out_ap=bc, in_ap=rec)
                ast = ast_pool.tile([64, 256], BF16, tag="ast", name="ast")
                nc.vector.tensor_mul(out=ast, in0=pvs[0:64, :], in1=bc)
                nc.sync.dma_start(
                    out=attds[s][hp * 128 + e * 64:hp * 128 + (e + 1) * 64, :],
                    in_=ast)
            if proj_ts and hp % 4 == 3:
                t = proj_ts[hp // 4]
                acts, xh = load_proj_inputs(t)
                for n in range(2):
                    proj_unit(t, n, acts, xh)

    attn_slot(0, None)
    # w_proj load deferred past slot0 so the mask/attds DMAs win the (serial)
    # DMA engine at attention start; first use is mid-slot1
    nc.scalar.dma_start(
        out=wpB, in_=io["w_proj"].rearrange("(c p) n -> p c n", p=128))
    attn_slot(1, [0, 1])
    attn_slot(2, [2, 3])
    attn_slot(3, [4, 5])

    # proj t6..7
    for t in range(6, 8):
        acts, xh = load_proj_inputs(t)
        for n in range(2):
            proj_unit(t, n, acts, xh)

    cm_ppp.__exit__(None, None, None)
    cm_pva_.__exit__(None, None, None)
    cm_pssc.__exit__(None, None, None)
    cm_pvs.__exit__(None, None, None)
    cm_ast.__exit__(None, None, None)
    cm_pt.__exit__(None, None, None)
    cm_att.__exit__(None, None, None)
    cm_wp.__exit__(None, None, None)
    cm_masks.__exit__(None, None, None)
    cm_qt.__exit__(None, None, None)
    cm_v.__exit__(None, None, None)
    cm_kt.__exit__(None, None, None)

    # ---------------- Phase 6: FFN ------------------------------------------
    cm_ls, late_singles = pool("lsing", 1)
    b2_sb = late_singles.tile([128, 1024], F32, name="b2_sb")
    nc.gpsimd.dma_start(out=b2_sb, in_=bcast_ap(io["b2"]))
    cm_wb, wbig_pool = pool("wbig", 3)

    def load_w1b(jb):
        w1b = wbig_pool.tile([128, 8, 1024], BF16, tag="wb", name="w1b")
        for hh in range(4):
            nc.sync.dma_start(
                out=w1b[:, hh * 2:(hh + 1) * 2, :],
                in_=io["w1"][hh * 256:(hh + 1) * 256,
                             jb * 1024:(jb + 1) * 1024]
                .rearrange("(c p) n -> p c n", p=128))
        return w1b

    def load_w2b(jb):
        w2b = wbig_pool.tile([128, 8, 1024], BF16, tag="wb", name="w2b")
        for hh in range(4):
            nc.sync.dma_start(
                out=w2b[:, hh * 2:(hh + 1) * 2, :],
                in_=io["w2"][jb * 1024 + hh * 256:jb * 1024 + (hh + 1) * 256, :]
                .rearrange("(j p) n -> p j n", p=128))
        return w2b

    w1b_next = load_w1b(0)
    w2b_next = load_w2b(0)

    cm_psl, ps_late = pool("ps_late", 5, "PSUM")
    # prefetch the FFN residual (x2) tiles; oacc is initialized from them
    cm_xr, xr_pool = pool("xres", 8)
    xres = [xr_pool.tile([128, 1024], BF16, tag="xr", name="xr")
            for _ in range(8)]
    for tg in range(8):
        nc.sync.dma_start(out=xres[tg], in_=x2d[tg * 128:(tg + 1) * 128, :])

    cm_rl, relu_pool = pool("relu", 2)
    cm_oa, oacc_pool = pool("oacc", 8)
    oacc = [oacc_pool.tile([128, 1024], F32, tag="oacc", name="oacc")
            for _ in range(8)]
    for jb in range(4):
        w1b = w1b_next
        relu_b = relu_pool.tile([128, 8, 2, 512], BF16, tag="rl", name="rl")
        for pas in range(2):
            tok0 = pas * 512
            for j in range(8):
                ps = ps_late.tile([128, 512], F32, tag="l", name="psf1")
                for c in range(8):
                    nc.tensor.matmul(
                        out=ps,
                        lhsT=w1b[:, c, j * 128:(j + 1) * 128],
                        rhs=h2T[:, c, tok0:tok0 + 512],
                        start=(c == 0), stop=(c == 7))
                nc.scalar.activation(
                    out=relu_b[:, j, pas, :], in_=ps, func=AF.Relu,
                    bias=b1t_sb[:, jb * 8 + j:jb * 8 + j + 1], scale=1.0)
        w2b = w2b_next
        if jb < 3:
            w1b_next = load_w1b(jb + 1)
        for pas in range(2):
            for tl in range(4):
                tg = pas * 4 + tl
                for n in range(2):
                    ps = ps_late.tile([128, 512], F32, tag="l", name="psf2")
                    for j in range(8):
                        nc.tensor.matmul(
                            out=ps,
                            lhsT=relu_b[:, j, pas, tl * 128:(tl + 1) * 128],
                            rhs=w2b[:, j, n * 512:(n + 1) * 512],
                            start=(j == 0), stop=(j == 7))
                    sl = np.s_[:, n * 512:(n + 1) * 512]
                    if jb == 0:
                        nc.vector.tensor_add(out=oacc[tg][sl], in0=ps,
                                             in1=xres[tg][sl])
                    else:
                        nc.vector.tensor_add(out=oacc[tg][sl], in0=oacc[tg][sl],
                                             in1=ps)
                if jb == 3:
                    nc.vector.tensor_add(out=oacc[tg], in0=oacc[tg], in1=b2_sb)
                    nc.sync.dma_start(out=io["out"][tg * 128:(tg + 1) * 128, :],
                                      in_=oacc[tg])
            if jb < 3 and pas == 0:
                w2b_next = load_w2b(jb + 1)

    cm_oa.__exit__(None, None, None)
    cm_rl.__exit__(None, None, None)
    cm_xr.__exit__(None, None, None)
    cm_psl.__exit__(None, None, None)
    cm_wb.__exit__(None, None, None)
    cm_ls.__exit__(None, None, None)
    cm_h2t.__exit__(None, None, None)
    cm_dram.__exit__(None, None, None)
    cm_small.__exit__(None, None, None)
    cm_stat.__exit__(None, None, None)
    cm_ln.__exit__(None, None, None)
    cm_singles.__exit__(None, None, None)


def build(ln1_triv=True, ln2_triv=True):
    key = (ln1_triv, ln2_triv)
    if key in _CACHE:
        return _CACHE[key]
    nc = bacc.Bacc("TRN2", target_bir_lowering=False, debug=False,
                   num_devices=NCORES)
    io = {}

    def din(name, shape, dt):
        io[name] = nc.dram_tensor(name, list(shape), dt, kind="ExternalInput").ap()

    din("x_full", (2048, 1024), BF16)
    din("x_half", (1024, 1024), BF16)
    din("wqk", (1024, 2048), BF16)
    din("wv", (1024, 1024), BF16)
    din("w_proj", (1024, 1024), BF16)
    din("b_proj", (1024,), F32)
    din("w1", (1024, 4096), BF16)
    din("b1t", (128, 32), F32)
    din("w2", (4096, 1024), BF16)
    din("b2", (1024,), F32)
    din("masks", (128, 16, 256), BF16)
    if not ln1_triv:
        din("ln1_g", (1024,), F32)
        din("ln1_b", (1024,), F32)
    if not ln2_triv:
        din("ln2_g", (1024,), F32)
        din("ln2_b", (1024,), F32)
    io["out"] = nc.dram_tensor("out", [1024, 1024], F32, kind="ExternalOutput").ap()

    with tile.TileContext(nc) as tc:
        _emit_body(nc, tc, io, ln1_triv, ln2_triv)
    nc.compile()
    _CACHE[key] = (nc, io)
    return nc, io


def _chunk_starts(half):
    # slot s (0..3) holds queries [512s+256*half, 512s+256*half+256)
    return [512 * s + 256 * half for s in range(4)]


def _make_masks(half):
    """[128, 16, 256] bf16: tile m = key tile 4s+j of slot s=m//4."""
    starts = _chunk_starts(half)
    out = np.zeros((128, 16, 256), np.float32)
    tk_l = np.arange(128)[:, None]
    tq_l = np.arange(256)[None, :]
    for m in range(16):
        q0 = starts[m // 4]
        out[:, m, :] = ((m * 128 + tk_l) <= (q0 + tq_l))
    return out.astype(ml_dtypes.bfloat16)


def _prep_common(inp, ln1_triv, ln2_triv):
    wq_f = np.ascontiguousarray(inp["wq"].transpose(1, 0, 2).reshape(C, C))
    wk_f = np.ascontiguousarray(inp["wk"].transpose(1, 0, 2).reshape(C, C))
    wv_f = np.ascontiguousarray(inp["wv"].transpose(1, 0, 2).reshape(C, C))
    wqk = np.concatenate([wq_f, wk_f], axis=1).astype(ml_dtypes.bfloat16)
    b1t = np.ascontiguousarray(inp["b1"].reshape(32, 128).T).astype(np.float32)
    common = {
        "wqk": wqk,
        "wv": wv_f.astype(ml_dtypes.bfloat16),
        "w_proj": inp["w_proj"].astype(ml_dtypes.bfloat16),
        "b_proj": inp["b_proj"].astype(np.float32),
        "w1": inp["w1"].astype(ml_dtypes.bfloat16),
        "b1t": b1t,
        "w2": inp["w2"].astype(ml_dtypes.bfloat16),
        "b2": inp["b2"].astype(np.float32),
    }
    if not ln1_triv:
        common["ln1_g"] = inp["ln1_g"].astype(np.float32)
        common["ln1_b"] = inp["ln1_b"].astype(np.float32)
    if not ln2_triv:
        common["ln2_g"] = inp["ln2_g"].astype(np.float32)
        common["ln2_b"] = inp["ln2_b"].astype(np.float32)
    return common


def make_in_maps(inputs):
    inp = {k: np.asarray(v) for k, v in inputs.items()}
    x = inp["x"].astype(np.float32)
    ln1_triv = bool(np.all(inp["ln1_g"] == 1.0) and np.all(inp["ln1_b"] == 0.0))
    ln2_triv = bool(np.all(inp["ln2_g"] == 1.0) and np.all(inp["ln2_b"] == 0.0))
    common = _prep_common(inp, ln1_triv, ln2_triv)
    in_maps = []
    for i in range(NCORES):
        b, half = i // 2, i % 2
        xh = np.concatenate(
            [x[b, st:st + 256] for st in _chunk_starts(half)], axis=0)
        m = dict(common)
        m["x_full"] = np.ascontiguousarray(x[b]).astype(ml_dtypes.bfloat16)
        m["x_half"] = np.ascontiguousarray(xh).astype(ml_dtypes.bfloat16)
        m["masks"] = _make_masks(half)
        in_maps.append(m)
    return in_maps, ln1_triv, ln2_triv


def assemble(results):
    out = np.empty((B, T, C), np.float32)
    for i in range(NCORES):
        b, half = i // 2, i % 2
        o = results[i]["out"]
        for s, st in enumerate(_chunk_starts(half)):
            out[b, st:st + 256] = o[256 * s:256 * (s + 1)]
    return out


def kernel(**inputs):
    in_maps, l1, l2 = make_in_maps(inputs)
    nc, io = build(l1, l2)
    res = run_bass_kernel_spmd(nc, in_maps, list(range(NCORES)))
    return assemble(res.results)


if __name__ == "__main__":
    build()
    print("build ok")



# revision 74
# speedup vs baseline: 1.1490x; 1.0256x over previous
"""Trainium2 Bass kernel for a dense transformer block (B=4,T=2048,C=1024,H=16).

Sharding: 8 cores, zero collectives. Core i handles batch i//2; its 1024
query tokens are four 256-token chunks, one per causal "slot" s=0..3 at
tokens [512s+256*(i%2), +256).  Slot s attends to exactly 512(s+1) keys, so
the uniform SPMD program does the causally-minimal score work (40 key-tile
passes per core vs 48 for a 2x512 split); per-core host-built multiplicative
masks handle the diagonal and the core asymmetry.  All sharding on the host;
the program is identical on every core, only input data differs.

Per-core dataflow (tokens-on-free-axis for all matmul operands):
  x ships bf16.  LN1 (bn_stats) -> h1 bf16 -> PE-transpose (no DRAM bounce)
  -> h1T [C, tok] in SBUF.  QKV bf16: kT [H*64, tok] (2 heads per tile); V
  natural [tok, H*65] with a fused ones column so PV also produces the
  softmax denominator; queries go to qTz [128, slot, 512], a zero-padded
  block-diagonal layout so ONE [128,512] matmul per key tile scores both
  stacked heads (full-rate, bank-exclusive PSUM).  wq is preloaded into the
  h2T tile (same shape, dead until mid-attention).  Scores are pre-
  transposed sT[tk, tq] so softmax needs no transpose of P and no max
  subtraction; exp runs on ACT straight from PSUM with the 1/sqrt(hd) scale
  fused.  V tiles 8-15 are deferred as PE fillers inside attention slots
  0-1.  After each slot, the proj for its two 128-token tiles runs fused:
  out-proj matmul + residual (+bias) in bf16, then LN2 + PE-transpose into
  h2T immediately - no separate LN2 phase.  FFN is jb-outer (weights load
  ONCE, bf16, double-buffered), jb0 initializes the accumulator with the
  prefetched x2 residual; all matmuls bf16 with fp32 PSUM accumulation.
  After each head-pair a single DVE copy moves the attention
  accumulator out of PSUM so the recip/broadcast/normalize chain runs
  off the PE critical path.  Measured (cost-model timeline): ~636us/core.
"""

import sys
import numpy as np

for _p in ("/opt/trn_rl_repo", "/root/.axon_site/_ro/trn_rl_repo"):
    if _p not in sys.path:
        sys.path.append(_p)

import ml_dtypes  # noqa: E402
import concourse.bass as bass  # noqa: E402
import concourse.bacc as bacc  # noqa: E402
import concourse.tile as tile  # noqa: E402
from concourse import mybir  # noqa: E402
from concourse.bass_utils import run_bass_kernel_spmd  # noqa: E402
from concourse.masks import make_identity  # noqa: E402

B, T, C, H, HD = 4, 2048, 1024, 16, 64
NCORES = 8
EPS = 1e-5
F32 = mybir.dt.float32
F32R = mybir.dt.float32r
BF16 = mybir.dt.bfloat16
AF = mybir.ActivationFunctionType
ALU = mybir.AluOpType

_CACHE = {}

def _emit_body(nc, tc, io, ln1_triv, ln2_triv):
    # ---------------- long-lived pools ----------------
    def pool(name, bufs, space="SBUF"):
        cm = tc.tile_pool(name=name, bufs=bufs, space=space)
        p = cm.__enter__()
        return cm, p

    cm_singles, singles = pool("singles", 1)
    cm_ln, ln_pool = pool("ln", 3)
    cm_stat, stat_pool = pool("stat", 4)
    cm_small, small = pool("small", 2)
    cm_dram, dram = pool("dram", 1, "DRAM")

    eps_t = singles.tile([128, 1], F32, name="eps")
    nc.vector.memset(eps_t, EPS)
    ident_bf = singles.tile([128, 128], BF16, name="ident_bf")
    make_identity(nc, ident_bf)
    b1t_sb = singles.tile([128, 32], F32, name="b1t_sb")
    nc.gpsimd.dma_start(out=b1t_sb, in_=io["b1t"])

    def bcast_ap(dram_ap):
        # [1024] dram vector -> [128,1024] partition-broadcast AP
        return bass.AP(
            tensor=dram_ap.tensor,
            offset=dram_ap.offset,
            ap=[[0, 128]] + list(dram_ap.ap),
        )

    bproj_sb = singles.tile([128, 1024], F32, name="bproj_sb")
    nc.gpsimd.dma_start(out=bproj_sb, in_=bcast_ap(io["b_proj"]))

    g1_sb = bb1_sb = g2_sb = bb2_sb = None
    if not ln1_triv:
        g1_sb = singles.tile([128, 1024], F32, name="g1_sb")
        nc.gpsimd.dma_start(out=g1_sb, in_=bcast_ap(io["ln1_g"]))
        bb1_sb = singles.tile([128, 1024], F32, name="bb1_sb")
        nc.gpsimd.dma_start(out=bb1_sb, in_=bcast_ap(io["ln1_b"]))
    if not ln2_triv:
        g2_sb = singles.tile([128, 1024], F32, name="g2_sb")
        nc.gpsimd.dma_start(out=g2_sb, in_=bcast_ap(io["ln2_g"]))
        bb2_sb = singles.tile([128, 1024], F32, name="bb2_sb")
        nc.gpsimd.dma_start(out=bb2_sb, in_=bcast_ap(io["ln2_b"]))

    # ---------------- LayerNorm helpers ----------------
    def ln_apply(xt, out_ap, trivial, g_sb, b_sb):
        st = stat_pool.tile([128, 2, 6], F32, tag="bnst", name="bnst")
        for sg in range(2):
            nc.vector.bn_stats(out=st[:, sg, :], in_=xt[:, sg * 512:(sg + 1) * 512])
        mv = stat_pool.tile([128, 2], F32, tag="bnmv", name="bnmv")
        nc.vector.bn_aggr(out=mv, in_=st)
        std = stat_pool.tile([128, 1], F32, tag="bnsd", name="bnsd")
        nc.scalar.activation(out=std, in_=mv[:, 1:2], func=AF.Sqrt, bias=eps_t,
                             scale=1.0)
        rstd = stat_pool.tile([128, 1], F32, tag="bnrs", name="bnrs")
        nc.vector.reciprocal(out=rstd, in_=std)
        if trivial:
            nc.vector.tensor_scalar(
                out=out_ap, in0=xt, scalar1=mv[:, 0:1], scalar2=rstd,
                op0=ALU.subtract, op1=ALU.mult)
        else:
            tmp = ln_pool.tile([128, 1024], F32, tag="lnx", name="lntmp")
            nc.vector.tensor_scalar(
                out=tmp, in0=xt, scalar1=mv[:, 0:1], scalar2=rstd,
                op0=ALU.subtract, op1=ALU.mult)
            nc.vector.tensor_mul(out=tmp, in0=tmp, in1=g_sb)
            nc.vector.tensor_add(out=out_ap, in0=tmp, in1=b_sb)

    def ln_tile(x_src_rows, out_ap, trivial, g_sb, b_sb):
        xt = ln_pool.tile([128, 1024], BF16, tag="lnx", name="lnx")
        nc.sync.dma_start(out=xt, in_=x_src_rows)
        ln_apply(xt, out_ap, trivial, g_sb, b_sb)

    # long-lived result pools, opened bottom-of-stack (LIFO discipline)
    cm_h2t, h2t_pool = pool("h2t", 1)
    h2T = h2t_pool.tile([128, 8, 1024], BF16, name="h2T")
    cm_kt, kt_pool = pool("kt", 8)
    cm_v, v_pool = pool("v", 16)
    cm_qt, qt_pool = pool("qt", 8)
    kT = [kt_pool.tile([128, 2048], BF16, tag="kt", name="kt") for _ in range(8)]
    Vt = [v_pool.tile([128, 16, 65], BF16, tag="vt", name="vt") for _ in range(16)]
    # qTz[p]: [128, 4(slot), 512] zero-padded block-diagonal queries: rows
    # 0-63 hold head-e0 q in cols 0-255, rows 64-127 head-e1 q in cols
    # 256-511.  One score matmul per key tile then covers both heads with
    # the full 128-partition contraction.
    qTz = [qt_pool.tile([128, 4, 512], BF16, tag="qt", name="qtz")
           for _ in range(8)]
    for p in range(8):
        nc.gpsimd.memset(qTz[p], 0.0)
    x2d = dram.tile([1024, 1024], BF16, name="x2d")

    cm_masks, masks_pool = pool("masks", 1)
    masks_sb = masks_pool.tile([128, 16, 256], BF16, name="masks_sb")
    nc.gpsimd.dma_start(out=masks_sb, in_=io["masks"])

    # ---------------- Phase 1+2: LN1 -> PE-transpose -> QKV -----------------
    # weight loads are emitted AFTER the first x tiles: the DMA engine is a
    # serial resource and the first LN tiles are on the critical path
    cm_wqkv, wqkv_pool = pool("wqkv", 2)
    wkB = wqkv_pool.tile([128, 8, 1024], BF16, tag="w", name="wkB")
    wvB = wqkv_pool.tile([128, 8, 1024], BF16, tag="w", name="wvB")

    cm_h1t, h1t_pool = pool("h1t", 2)
    cm_pst, ps_tr = pool("ps_tr", 4, "PSUM")
    cm_psq, ps_qkv = pool("ps_qkv", 4, "PSUM")
    cm_h1, h1_pool = pool("h1", 2)

    def ln_transpose(src_rows, dstT, col):
        ht = h1_pool.tile([128, 1024], BF16, tag="h1", name="h1")
        ln_tile(src_rows, ht, ln1_triv, g1_sb, bb1_sb)
        for g in range(2):
            pst = ps_tr.tile([128, 4, 128], BF16, tag="tr", name="pst")
            for c4 in range(4):
                nc.tensor.transpose(
                    out=pst[:, c4, :],
                    in_=ht[:, (g * 4 + c4) * 128:(g * 4 + c4 + 1) * 128],
                    identity=ident_bf)
            nc.scalar.copy(
                out=dstT[:, g * 4:(g + 1) * 4, col * 128:(col + 1) * 128],
                in_=pst)

    h1TA = h1t_pool.tile([128, 8, 1024], BF16, tag="h1t", name="h1TA")
    h1TB = h1t_pool.tile([128, 8, 1024], BF16, tag="h1t", name="h1TB")

    def h1T(c, n):
        # transposed h1 slice [128, 512] for token chunk n (0..3)
        src = h1TA if n < 2 else h1TB
        return src[:, c, (n % 2) * 512:(n % 2 + 1) * 512]

    def h1Tt(c, t):
        # transposed h1 slice [128, 128] for token tile t (0..15)
        src = h1TA if t < 8 else h1TB
        return src[:, c, (t % 8) * 128:(t % 8 + 1) * 128]

    def kt_unit(n, p):
        ps = ps_qkv.tile([128, 512], F32, tag="q", name="psk")
        for c in range(8):
            nc.tensor.matmul(
                out=ps, lhsT=wkB[:, c, p * 128:(p + 1) * 128],
                rhs=h1T(c, n), start=(c == 0), stop=(c == 7))
        nc.vector.tensor_copy(out=kT[p][:, n * 512:(n + 1) * 512], in_=ps)

    def v_unit(t, n):
        ps = ps_qkv.tile([128, 512], F32, tag="q", name="psv")
        for c in range(8):
            nc.tensor.matmul(
                out=ps, lhsT=h1Tt(c, t),
                rhs=wvB[:, c, n * 512:(n + 1) * 512],
                start=(c == 0), stop=(c == 7))
        nc.vector.tensor_copy(
            out=Vt[t][:, n * 8:(n + 1) * 8, 0:64],
            in_=ps.rearrange("p (h d) -> p h d", d=64))
        if n == 1:
            nc.vector.memset(Vt[t][:, :, 64:65], 1.0)

    def q_unit(n, p):
        ps = ps_qkv.tile([128, 512], F32, tag="q", name="psq")
        for c in range(8):
            nc.tensor.matmul(
                out=ps, lhsT=h2T[:, c, p * 128:(p + 1) * 128],
                rhs=h1Th[:, c, n * 512:(n + 1) * 512],
                start=(c == 0), stop=(c == 7))
        for sh in range(2):
            s = 2 * n + sh
            nc.vector.tensor_copy(out=qTz[p][0:64, s, 0:256],
                                  in_=ps[0:64, sh * 256:(sh + 1) * 256])
            nc.vector.tensor_copy(out=qTz[p][64:128, s, 256:512],
                                  in_=ps[64:128, sh * 256:(sh + 1) * 256])

    # A half (tokens 0-1023): LN+transpose, then K(n=0,1) and V tiles 0-7
    for t in range(8):
        ln_transpose(io["x_full"][t * 128:(t + 1) * 128, :], h1TA, t)
        if t == 3:
            nc.sync.dma_start(
                out=wkB,
                in_=io["wqk"][:, 1024:2048].rearrange("(c p) n -> p c n", p=128))
        if t == 4:
            nc.sync.dma_start(
                out=wvB, in_=io["wv"].rearrange("(c p) n -> p c n", p=128))
        if t == 5:
            nc.sync.dma_start(
                out=h2T,
                in_=io["wqk"][:, 0:1024].rearrange("(c p) n -> p c n", p=128))
        if 4 <= t:
            kt_unit(0, 2 * (t - 4))
            kt_unit(0, 2 * (t - 4) + 1)
    for t in range(8):
        v_unit(t, 0)
        v_unit(t, 1)
        if t % 2 == 1:
            kt_unit(1, t - 1)
            kt_unit(1, t)
    # B half (tokens 1024-2047): transposes, then K(n=2,3)
    for t in range(8, 16):
        ln_transpose(io["x_full"][t * 128:(t + 1) * 128, :], h1TB, t - 8)
        if t == 15:
            for p in range(8):
                kt_unit(2, p)
    for p in range(8):
        kt_unit(3, p)
    # queries: LN+transpose of x_half; V tiles 8-15 run on PE while the
    # x_half tiles stream in.  wq was preloaded into h2T (same shape, not
    # written until the slot-1 proj), so q units need no weight wait.
    h1Th = h1t_pool.tile([128, 8, 1024], BF16, tag="h1t", name="h1Th")
    for th in range(8):
        ln_transpose(io["x_half"][th * 128:(th + 1) * 128, :], h1Th, th)
        v_unit(8 + th, 0)
        v_unit(8 + th, 1)
    for n in range(2):
        for p in range(8):
            q_unit(n, p)
    cm_h1.__exit__(None, None, None)
    cm_psq.__exit__(None, None, None)
    cm_pst.__exit__(None, None, None)
    cm_h1t.__exit__(None, None, None)
    cm_wqkv.__exit__(None, None, None)

    # ---------------- Phase 3: attention (sw-pipelined) ---------------------
    cm_wp, wp_pool = pool("wproj", 1)
    wpB = wp_pool.tile([128, 8, 1024], BF16, name="wpB")
    cm_att, att_pool = pool("attls", 2)

    def mask_b(m):
        # [128, 2(e), 256] view of mask tile m, broadcast over the e axis
        ap = masks_sb[:, m, :]
        return bass.AP(tensor=ap.tensor, offset=ap.offset,
                       ap=[list(ap.ap[0]), [0, 2], list(ap.ap[1])])

    cm_pt, pt_pool = pool("pt", 3)
    cm_ast, ast_pool = pool("attst", 2)
    cm_pvs, pvs_pool = pool("pvs", 4)
    cm_pssc, ps_sc = pool("ps_sc", 2, "PSUM")
    cm_pva_, ps_pva = pool("ps_pva", 2, "PSUM")
    cm_ppp, ps_pp = pool("ps_pp", 1, "PSUM")

    attds = [dram.tile([1024, 256], BF16, name="attd")
             for _ in range(4)]  # [c=h*64+d, 256 tq] per slot
    SCALE = HD ** -0.5

    # proj units (t, n): t-tiles 2s,2s+1 available after slot s.
    # n==1 fuses LN2 + PE-transpose of the finished x2 tile into h2T.
    def proj_unit(t, n, acts, xh):
        ps = ps_pp.tile([128, 512], F32, tag="pp", name="psp")
        for c in range(8):
            nc.tensor.matmul(
                out=ps, lhsT=acts[:, c, :],
                rhs=wpB[:, c, n * 512:(n + 1) * 512],
                start=(c == 0), stop=(c == 7))
        sl = np.s_[:, n * 512:(n + 1) * 512]
        x2t = ln_pool.tile([128, 1024], BF16, tag="x2b", name="x2t") \
            if n == 0 else proj_unit.x2t
        proj_unit.x2t = x2t
        with nc.allow_low_precision(reason="x2 residual kept in bf16"):
            nc.vector.tensor_add(out=x2t[sl], in0=ps, in1=xh[sl])
            nc.vector.tensor_add(out=x2t[sl], in0=x2t[sl], in1=bproj_sb[sl])
        if n == 1:
            nc.sync.dma_start(out=x2d[t * 128:(t + 1) * 128, :], in_=x2t)
            h2 = ln_pool.tile([128, 1024], BF16, tag="lnx", name="h2")
            ln_apply(x2t, h2, ln2_triv, g2_sb, bb2_sb)
            for g in range(2):
                pst = ps_pp.tile([128, 4, 128], BF16, tag="tr2", name="pst2")
                for c4 in range(4):
                    nc.tensor.transpose(
                        out=pst[:, c4, :],
                        in_=h2[:, (g * 4 + c4) * 128:(g * 4 + c4 + 1) * 128],
                        identity=ident_bf)
                nc.scalar.copy(
                    out=h2T[:, g * 4:(g + 1) * 4, t * 128:(t + 1) * 128],
                    in_=pst)

    def load_proj_inputs(t):
        acts = att_pool.tile([128, 8, 128], BF16, tag="attls", name="attls")
        nc.scalar.dma_start(
            out=acts,
            in_=attds[t // 2][:, (t % 2) * 128:(t % 2 + 1) * 128]
            .rearrange("(c p) n -> p c n", p=128))
        xh = ln_pool.tile([128, 1024], BF16, tag="lnx", name="xh2")
        nc.sync.dma_start(out=xh, in_=io["x_half"][t * 128:(t + 1) * 128, :])
        return acts, xh

    def attn_slot(s, proj_ts):
        ntk = 4 * (s + 1)
        npair = ntk // 2
        qc = 256 * s
        for hp in range(8):
            pva = [ps_pva.tile([128, 512], F32, tag="pv", name="pv")
                   for _ in range(2)]
            pts = {}
            for j in range(npair):
                ps = ps_sc.tile([128, 2, 512], F32, tag="sc", name="sc")
                for tk2 in range(2):
                    nc.tensor.matmul(
                        out=ps[:, tk2, :],
                        lhsT=kT[hp][:, (2 * j + tk2) * 128:
                                    (2 * j + tk2 + 1) * 128],
                        rhs=qTz[hp][:, s, :],
                        start=True, stop=True)
                pt = pt_pool.tile([128, 2, 2, 256], BF16, tag="pt", name="pt")
                pts[j] = pt
                nc.scalar.activation(
                    out=pt.rearrange("p a b c -> p (a b c)"),
                    in_=ps.rearrange("p a b -> p (a b)"),
                    func=AF.Exp, scale=SCALE)
                if j >= npair - 2:
                    for tk2 in range(2):
                        m = 4 * s + 2 * (j - (npair - 2)) + tk2
                        for e in range(2):
                            nc.vector.tensor_mul(
                                out=pt[:, tk2, e, :], in0=pt[:, tk2, e, :],
                                in1=masks_sb[:, m, :])
                if j >= 1:
                    prev = pts.pop(j - 1)
                    for tk2 in range(2):
                        for e in range(2):
                            nc.tensor.matmul(
                                out=pva[e][0:65, 0:256],
                                lhsT=Vt[2 * (j - 1) + tk2][:, 2 * hp + e, :],
                                rhs=prev[:, tk2, e, :],
                                start=(j == 1 and tk2 == 0), stop=False)
            last = pts.pop(npair - 1)
            for tk2 in range(2):
                for e in range(2):
                    nc.tensor.matmul(
                        out=pva[e][0:65, 0:256],
                        lhsT=Vt[ntk - 2 + tk2][:, 2 * hp + e, :],
                        rhs=last[:, tk2, e, :],
                        start=False, stop=(tk2 == 1))
            for e in range(2):
                # one fast copy frees the pva psum bank for the next head
                # pair; the recip/broadcast/normalize chain runs off-path
                pvs = pvs_pool.tile([65, 256], BF16, tag="pvs", name="pvs")
                with nc.allow_low_precision(reason="attn out normalized bf16"):
                    nc.vector.tensor_copy(out=pvs, in_=pva[e][0:65, 0:256])
                rec = small.tile([1, 256], BF16, tag="rec", name="rec")
                with nc.allow_low_precision(reason="softmax denom recip bf16"):
                    nc.vector.reciprocal(out=rec, in_=pvs[64:65, :])
                bc = small.tile([64, 256], BF16, tag="bc", name="bc")
                nc.gpsimd.partition_broadcast(out_ap=bc, in_ap=rec)
                ast = ast_pool.tile([64, 256], BF16, tag="ast", name="ast")
                nc.vector.tensor_mul(out=ast, in0=pvs[0:64, :], in1=bc)
                nc.sync.dma_start(
                    out=attds[s][hp * 128 + e * 64:hp * 128 + (e + 1) * 64, :],
                    in_=ast)
            if proj_ts and hp % 4 == 3:
                t = proj_ts[hp // 4]
                acts, xh = load_proj_inputs(t)
                for n in range(2):
                    proj_unit(t, n, acts, xh)

    attn_slot(0, None)
    # w_proj load deferred past slot0 so the mask/attds DMAs win the (serial)
    # DMA engine at attention start; first use is mid-slot1
    nc.scalar.dma_start(
        out=wpB, in_=io["w_proj"].rearrange("(c p) n -> p c n", p=128))
    attn_slot(1, [0, 1])
    attn_slot(2, [2, 3])
    attn_slot(3, [4, 5])

    # proj t6..7
    for t in range(6, 8):
        acts, xh = load_proj_inputs(t)
        for n in range(2):
            proj_unit(t, n, acts, xh)

    cm_ppp.__exit__(None, None, None)
    cm_pva_.__exit__(None, None, None)
    cm_pssc.__exit__(None, None, None)
    cm_pvs.__exit__(None, None, None)
    cm_ast.__exit__(None, None, None)
    cm_pt.__exit__(None, None, None)
    cm_att.__exit__(None, None, None)
    cm_wp.__exit__(None, None, None)
    cm_masks.__exit__(None, None, None)
    cm_qt.__exit__(None, None, None)
    cm_v.__exit__(None, None, None)
    cm_kt.__exit__(None, None, None)

    # ---------------- Phase 6: FFN ------------------------------------------
    cm_ls, late_singles = pool("lsing", 1)
    b2_sb = late_singles.tile([128, 1024], F32, name="b2_sb")
    nc.gpsimd.dma_start(out=b2_sb, in_=bcast_ap(io["b2"]))
    cm_wb, wbig_pool = pool("wbig", 3)

    def load_w1b(jb):
        w1b = wbig_pool.tile([128, 8, 1024], BF16, tag="wb", name="w1b")
        for hh in range(4):
            nc.sync.dma_start(
                out=w1b[:, hh * 2:(hh + 1) * 2, :],
                in_=io["w1"][hh * 256:(hh + 1) * 256,
                             jb * 1024:(jb + 1) * 1024]
                .rearrange("(c p) n -> p c n", p=128))
        return w1b

    def load_w2b(jb):
        w2b = wbig_pool.tile([128, 8, 1024], BF16, tag="wb", name="w2b")
        for hh in range(4):
            nc.sync.dma_start(
                out=w2b[:, hh * 2:(hh + 1) * 2, :],
                in_=io["w2"][jb * 1024 + hh * 256:jb * 1024 + (hh + 1) * 256, :]
                .rearrange("(j p) n -> p j n", p=128))
        return w2b

    w1b_next = load_w1b(0)
    w2b_next = load_w2b(0)

    cm_psl, ps_late = pool("ps_late", 5, "PSUM")
    # prefetch the FFN residual (x2) tiles; oacc is initialized from them
    cm_xr, xr_pool = pool("xres", 8)
    xres = [xr_pool.tile([128, 1024], BF16, tag="xr", name="xr")
            for _ in range(8)]
    for tg in range(8):
        nc.sync.dma_start(out=xres[tg], in_=x2d[tg * 128:(tg + 1) * 128, :])

    cm_rl, relu_pool = pool("relu", 2)
    cm_oa, oacc_pool = pool("oacc", 8)
    oacc = [oacc_pool.tile([128, 1024], F32, tag="oacc", name="oacc")
            for _ in range(8)]
    for jb in range(4):
        w1b = w1b_next
        relu_b = relu_pool.tile([128, 8, 2, 512], BF16, tag="rl", name="rl")
        for pas in range(2):
            tok0 = pas * 512
            for j in range(8):
                ps = ps_late.tile([128, 512], F32, tag="l", name="psf1")
                for c in range(8):
                    nc.tensor.matmul(
                        out=ps,
                        lhsT=w1b[:, c, j * 128:(j + 1) * 128],
                        rhs=h2T[:, c, tok0:tok0 + 512],
                        start=(c == 0), stop=(c == 7))
                nc.scalar.activation(
                    out=relu_b[:, j, pas, :], in_=ps, func=AF.Relu,
                    bias=b1t_sb[:, jb * 8 + j:jb * 8 + j + 1], scale=1.0)
        w2b = w2b_next
        if jb < 3:
            w1b_next = load_w1b(jb + 1)
        for pas in range(2):
            for tl in range(4):
                tg = pas * 4 + tl
                for n in range(2):
                    ps = ps_late.tile([128, 512], F32, tag="l", name="psf2")
                    for j in range(8):
                        nc.tensor.matmul(
                            out=ps,
                            lhsT=relu_b[:, j, pas, tl * 128:(tl + 1) * 128],
                            rhs=w2b[:, j, n * 512:(n + 1) * 512],
                            start=(j == 0), stop=(j == 7))
                    sl = np.s_[:, n * 512:(n + 1) * 512]
                    if jb == 0:
                        nc.vector.tensor_add(out=oacc[tg][sl], in0=ps,
                                             in1=xres[tg][sl])
                    else:
                        nc.vector.tensor_add(out=oacc[tg][sl], in0=oacc[tg][sl],
                                             in1=ps)
                if jb == 3:
                    nc.vector.tensor_add(out=oacc[tg], in0=oacc[tg], in1=b2_sb)
                    nc.sync.dma_start(out=io["out"][tg * 128:(tg + 1) * 128, :],
                                      in_=oacc[tg])
            if jb < 3 and pas == 0:
                w2b_next = load_w2b(jb + 1)

    cm_oa.__exit__(None, None, None)
    cm_rl.__exit__(None, None, None)
    cm_xr.__exit__(None, None, None)
    cm_psl.__exit__(None, None, None)
    cm_wb.__exit__(None, None, None)
    cm_ls.__exit__(None, None, None)
    cm_h2t.__exit__(None, None, None)
    cm_dram.__exit__(None, None, None)
    cm_small.__exit__(None, None, None)
    cm_stat.__exit__(None, None, None)
    cm_ln.__exit__(None, None, None)
    cm_singles.__exit__(None, None, None)


def build(ln1_triv=True, ln2_triv=True):
    key = (ln1_triv, ln2_triv)
    if key in _CACHE:
        return _CACHE[key]
    nc = bacc.Bacc("TRN2", target_bir_lowering=False, debug=False,
                   num_devices=NCORES)
    io = {}

    def din(name, shape, dt):
        io[name] = nc.dram_tensor(name, list(shape), dt, kind="ExternalInput").ap()

    din("x_full", (2048, 1024), BF16)
    din("x_half", (1024, 1024), BF16)
    din("wqk", (1024, 2048), BF16)
    din("wv", (1024, 1024), BF16)
    din("w_proj", (1024, 1024), BF16)
    din("b_proj", (1024,), F32)
    din("w1", (1024, 4096), BF16)
    din("b1t", (128, 32), F32)
    din("w2", (4096, 1024), BF16)
    din("b2", (1024,), F32)
    din("masks", (128, 16, 256), BF16)
    if not ln1_triv:
        din("ln1_g", (1024,), F32)
        din("ln1_b", (1024,), F32)
    if not ln2_triv:
        din("ln2_g", (1024,), F32)
        din("ln2_b", (1024,), F32)
    io["out"] = nc.dram_tensor("out", [1024, 1024], F32, kind="ExternalOutput").ap()

    with tile.TileContext(nc) as tc:
        _emit_body(nc, tc, io, ln1_triv, ln2_triv)
    nc.compile()
    _CACHE[key] = (nc, io)
    return nc, io


def _chunk_starts(half):
    # slot s (0..3) holds queries [512s+256*half, 512s+256*half+256)
    return [512 * s + 256 * half for s in range(4)]


def _make_masks(half):
    """[128, 16, 256] bf16: tile m = key tile 4s+j of slot s=m//4."""
    starts = _chunk_starts(half)
    out = np.zeros((128, 16, 256), np.float32)
    tk_l = np.arange(128)[:, None]
    tq_l = np.arange(256)[None, :]
    for m in range(16):
        q0 = starts[m // 4]
        out[:, m, :] = ((m * 128 + tk_l) <= (q0 + tq_l))
    return out.astype(ml_dtypes.bfloat16)


def _prep_common(inp, ln1_triv, ln2_triv):
    wq_f = np.ascontiguousarray(inp["wq"].transpose(1, 0, 2).reshape(C, C))
    wk_f = np.ascontiguousarray(inp["wk"].transpose(1, 0, 2).reshape(C, C))
    wv_f = np.ascontiguousarray(inp["wv"].transpose(1, 0, 2).reshape(C, C))
    wqk = np.concatenate([wq_f, wk_f], axis=1).astype(ml_dtypes.bfloat16)
    b1t = np.ascontiguousarray(inp["b1"].reshape(32, 128).T).astype(np.float32)
    common = {
        "wqk": wqk,
        "wv": wv_f.astype(ml_dtypes.bfloat16),
        "w_proj": inp["w_proj"].astype(ml_dtypes.bfloat16),
        "b_proj": inp["b_proj"].astype(np.float32),
        "w1": inp["w1"].astype(ml_dtypes.bfloat16),
        "b1t": b1t,
        "w2": inp["w2"].astype(ml_dtypes.bfloat16),
        "b2": inp["b2"].astype(np.float32),
    }
    if not ln1_triv:
        common["ln1_g"] = inp["ln1_g"].astype(np.float32)
        common["ln1_b"] = inp["ln1_b"].astype(np.float32)
    if not ln2_triv:
        common["ln2_g"] = inp["ln2_g"].astype(np.float32)
        common["ln2_b"] = inp["ln2_b"].astype(np.float32)
    return common


def make_in_maps(inputs):
    inp = {k: np.asarray(v) for k, v in inputs.items()}
    x = inp["x"].astype(np.float32)
    ln1_triv = bool(np.all(inp["ln1_g"] == 1.0) and np.all(inp["ln1_b"] == 0.0))
    ln2_triv = bool(np.all(inp["ln2_g"] == 1.0) and np.all(inp["ln2_b"] == 0.0))
    common = _prep_common(inp, ln1_triv, ln2_triv)
    in_maps = []
    for i in range(NCORES):
        b, half = i // 2, i % 2
        xh = np.concatenate(
            [x[b, st:st + 256] for st in _chunk_starts(half)], axis=0)
        m = dict(common)
        m["x_full"] = np.ascontiguousarray(x[b]).astype(ml_dtypes.bfloat16)
        m["x_half"] = np.ascontiguousarray(xh).astype(ml_dtypes.bfloat16)
        m["masks"] = _make_masks(half)
        in_maps.append(m)
    return in_maps, ln1_triv, ln2_triv


def assemble(results):
    out = np.empty((B, T, C), np.float32)
    for i in range(NCORES):
        b, half = i // 2, i % 2
        o = results[i]["out"]
        for s, st in enumerate(_chunk_starts(half)):
            out[b, st:st + 256] = o[256 * s:256 * (s + 1)]
    return out


def kernel(**inputs):
    in_maps, l1, l2 = make_in_maps(inputs)
    nc, io = build(l1, l2)
    res = run_bass_kernel_spmd(nc, in_maps, list(range(NCORES)))
    return assemble(res.results)


if __name__ == "__main__":
    build()
    print("build ok")

